# revision 3
# baseline (speedup 1.0000x reference)
"""GPTQ-Marlin sparse MoE layer for 8 Trainium2 NeuronCores.

Strategy (expert-parallel with overflow segments, host-side dispatch):
  - Router (softmax + top-2 + renormalize) replicated with the same jax ops
    as the reference so expert selection matches bit-for-bit.
  - Per-core uniform SPMD structure of two token segments:
      seg1: S1 (=1024) token slots running one "main" expert,
      seg2: S2 (=64)  token slots running an "overflow" expert.
    Expert e's first S1 tokens go to core e's seg1; tokens beyond S1 are
    split into <=S2-sized pieces placed on other cores' seg2 (with that
    expert's weights streamed there as a second weight set). This cuts the
    per-core token capacity from pad128(max_e n_e) (=1152 for the reference
    routing) to S1+S2 (=1088), which directly reduces PE streaming time --
    the kernel is bf16 tensor-engine bound.
  - GPTQ int4 codes are dequantized to bf16 on the host; each core streams
    its experts' W1 [D,2F] / W2 [F,D] from HBM.
  - Device kernel per core and segment: h = x @ W1 (transposed layout),
    act = silu(gate)*up, y = act @ W2 -- bf16 matmuls, fp32 PSUM.
  - Host applies the top-k coefficients during the scatter-add combine.

Layout (activations keep tokens on the free dim; no on-device transposes):
  mm1: psum[n, t] = sum_k W1[k, n] * xT[k, t]   (lhsT = W1 as stored)
  mm2: psum[d, t] = sum_f W2[f, d] * actT[f, t] (lhsT = W2 as stored)

Startup/gap optimizations vs the naive schedule:
  - First x / W1-chunk DMAs are split into sub-slices and issued from
    different engine queues (vector/sync/gpsimd) so the first matmul can
    start as soon as ~0.5 MiB has landed instead of ~4.4 MiB.
  - A dummy Silu activation at program start preloads the ACT engine's
    function table so the first real silu doesn't eat a ~1.3us table load.
  - mm1 uses 6 PSUM banks (3 in-flight gate/up pairs), mm2 the other 2.
  - The psum->sbuf copy for mm2 output runs on the vector engine so the
    scalar engine only ever runs Silu (no ACT table switching).
"""

import numpy as np
import ml_dtypes

E, T, D, F, TOPK, GROUP = 8, 4096, 1024, 4096, 2, 128
P = 128
KO1 = D // P           # 8  k-tiles for mm1
FH = F // P            # 32 act tiles (and k-tiles for mm2)
DO = D // P            # 8  output d-tiles
NCH = (2 * F) // 1024  # 8 column chunks of W1 (0..3 gate, 4..7 up)
TC = 512               # token chunk (one PSUM bank of fp32)

LAST_RESULTS = None    # test harness introspection

_BUILD_CACHE = {}


def _route(gating_output):
    """softmax + top-k + renormalize, replicated exactly like the reference."""
    try:
        import jax
        import jax.numpy as jnp

        scores = jax.nn.softmax(jnp.asarray(gating_output, jnp.float32), axis=-1)
        topk_w, topk_ids = jax.lax.top_k(scores, TOPK)
        topk_w = topk_w / jnp.sum(topk_w, axis=-1, keepdims=True)
        return np.asarray(topk_w, np.float32), np.asarray(topk_ids)
    except Exception:
        g = np.asarray(gating_output, np.float32)
        ex = np.exp(g - g.max(axis=-1, keepdims=True))
        s = (ex / ex.sum(axis=-1, keepdims=True)).astype(np.float32)
        ids = np.argsort(-s, axis=-1, kind="stable")[:, :TOPK]
        w = np.take_along_axis(s, ids, axis=-1)
        w = (w / w.sum(axis=-1, keepdims=True)).astype(np.float32)
        return w, ids


def _dequant_bf16(q, s):
    """q: [K, N] int codes, s: [K//GROUP, N] scales -> bf16 [K, N]."""
    w = (np.asarray(q, np.float32) - 8.0) * np.repeat(
        np.asarray(s, np.float32), GROUP, axis=0
    )
    return w.astype(ml_dtypes.bfloat16)


def _token_chunks(C):
    tcs, t0 = [], 0
    while t0 < C:
        w = min(TC, C - t0)
        tcs.append((t0, w))
        t0 += w
    return tcs


def _build(C1, C2, native_silu=True):
    """Per-core FFN program: seg1 of C1 tokens (expert A), seg2 of C2 tokens
    (expert B). C1/C2 are multiples of 64 (C2 may be 0)."""
    import concourse.mybir as mybir
    import concourse.tile as tile
    from concourse import bacc

    nc = bacc.Bacc("TRN2", name="moe_expert_ffn")
    bf16 = mybir.dt.bfloat16
    f32 = mybir.dt.float32

    segs = []
    xT1 = nc.dram_tensor("xT1", [P, KO1, C1], bf16, kind="ExternalInput")
    w1a = nc.dram_tensor("w1a", [P, NCH, KO1, 1024], bf16, kind="ExternalInput")
    w2a = nc.dram_tensor("w2a", [P, DO, FH, P], bf16, kind="ExternalInput")
    yT1 = nc.dram_tensor("yT1", [P, DO, C1], f32, kind="ExternalOutput")
    segs.append((xT1, w1a, w2a, yT1, C1, "act1", True))
    if C2:
        xT2 = nc.dram_tensor("xT2", [P, KO1, C2], bf16, kind="ExternalInput")
        w1b = nc.dram_tensor("w1b", [P, NCH, KO1, 1024], bf16, kind="ExternalInput")
        w2b = nc.dram_tensor("w2b", [P, DO, FH, P], bf16, kind="ExternalInput")
        yT2 = nc.dram_tensor("yT2", [P, DO, C2], f32, kind="ExternalOutput")
        segs.append((xT2, w1b, w2b, yT2, C2, "act2", False))

    with tile.TileContext(nc) as tc:
        with (
            tc.tile_pool(name="xpool", bufs=1) as xpool,
            tc.tile_pool(name="w1pool", bufs=4) as w1pool,
            tc.tile_pool(name="w2pool", bufs=3) as w2pool,
            tc.tile_pool(name="actpool", bufs=1) as actpool,
            tc.tile_pool(name="sgpool", bufs=4) as sgpool,
            tc.tile_pool(name="ypool", bufs=4) as ypool,
            tc.tile_pool(name="pspool", bufs=6, space="PSUM") as pspool,
            tc.tile_pool(name="psypool", bufs=2, space="PSUM") as psypool,
        ):
            for xT, w1, w2, yT, C, act_tag, first in segs:
                tcs = _token_chunks(C)

                xsb = xpool.tile([P, KO1, C], bf16, tag=act_tag + "x")
                if first:
                    # split so the first k-tiles land early; issue on the
                    # scalar queue to keep sync/gpsimd free for weights
                    nc.scalar.dma_start(xsb[:, 0:2], xT[:, 0:2])
                    nc.scalar.dma_start(xsb[:, 2:4], xT[:, 2:4])
                    nc.scalar.dma_start(xsb[:, 4:8], xT[:, 4:8])
                    # preload the ACT engine's Silu table during the DMA wait
                    warm_in = sgpool.tile([P, 8], f32, tag="warm", bufs=1)
                    warm_out = sgpool.tile([P, 8], f32, tag="warm2", bufs=1)
                    nc.gpsimd.memset(warm_in[:], 0.0)
                    if native_silu:
                        nc.scalar.activation(
                            warm_out[:], warm_in[:],
                            mybir.ActivationFunctionType.Silu,
                        )
                else:
                    nc.scalar.dma_start(xsb[:], xT[:])
                act = actpool.tile([P, FH, C], bf16, tag=act_tag)

                # ---- mm1: h^T = W1^T x, then act = silu(gate) * up ----
                for c in range(NCH // 2):
                    wg = w1pool.tile([P, KO1, 1024], bf16, tag="w1c")
                    wu = w1pool.tile([P, KO1, 1024], bf16, tag="w1c")
                    if first and c == 0:
                        # fine-grained first chunk: matmuls can start after
                        # ~0.5 MiB instead of 4 MiB
                        nc.sync.dma_start(wg[:, 0:2], w1[:, 0, 0:2])
                        nc.gpsimd.dma_start(wu[:, 0:2], w1[:, 4, 0:2])
                        nc.sync.dma_start(wg[:, 2:4], w1[:, 0, 2:4])
                        nc.gpsimd.dma_start(wu[:, 2:4], w1[:, 4, 2:4])
                        nc.sync.dma_start(wg[:, 4:8], w1[:, 0, 4:8])
                        nc.gpsimd.dma_start(wu[:, 4:8], w1[:, 4, 4:8])
                    else:
                        nc.sync.dma_start(wg[:], w1[:, c])
                        nc.gpsimd.dma_start(wu[:], w1[:, c + NCH // 2])
                    for j in range(8):
                        i = c * 8 + j  # act tile index 0..31
                        for (t0, tw) in tcs:
                            psg = pspool.tile([P, TC], f32, tag="ps")
                            psu = pspool.tile([P, TC], f32, tag="ps")
                            for k in range(KO1):
                                nc.tensor.matmul(
                                    psg[:, :tw],
                                    wg[:, k, j * P : (j + 1) * P],
                                    xsb[:, k, t0 : t0 + tw],
                                    start=(k == 0),
                                    stop=(k == KO1 - 1),
                                )
                                nc.tensor.matmul(
                                    psu[:, :tw],
                                    wu[:, k, j * P : (j + 1) * P],
                                    xsb[:, k, t0 : t0 + tw],
                                    start=(k == 0),
                                    stop=(k == KO1 - 1),
                                )
                            sg = sgpool.tile([P, TC], f32, tag="sg")
                            if native_silu:
                                nc.scalar.activation(
                                    sg[:, :tw], psg[:, :tw],
                                    mybir.ActivationFunctionType.Silu,
                                )
                            else:
                                nc.scalar.activation(
                                    sg[:, :tw], psg[:, :tw],
                                    mybir.ActivationFunctionType.Sigmoid,
                                )
                                nc.vector.tensor_tensor(
                                    sg[:, :tw],
                                    sg[:, :tw],
                                    psg[:, :tw],
                                    mybir.AluOpType.mult,
                                )
                            nc.vector.tensor_tensor(
                                act[:, i, t0 : t0 + tw],
                                sg[:, :tw],
                                psu[:, :tw],
                                mybir.AluOpType.mult,
                            )

                # ---- mm2: y^T = W2^T act ----
                for d in range(DO):
                    ws = w2pool.tile([P, FH, P], bf16, tag="w2s")
                    nc.gpsimd.dma_start(ws[:], w2[:, d])
                    for (t0, tw) in tcs:
                        psy = psypool.tile([P, TC], f32, tag="psy")
                        for k2 in range(FH):
                            nc.tensor.matmul(
                                psy[:, :tw],
                                ws[:, k2],
                                act[:, k2, t0 : t0 + tw],
                                start=(k2 == 0),
                                stop=(k2 == FH - 1),
                            )
                        yo = ypool.tile([P, TC], f32, tag="yo")
                        nc.vector.tensor_copy(yo[:, :tw], psy[:, :tw])
                        nc.sync.dma_start(yT[:, d, t0 : t0 + tw], yo[:, :tw])
    return nc


def _pack(counts, S1=1024):
    """Choose seg2 size S2 and the overflow piece placement.

    Returns (S2, pieces) where pieces[core] = (expert, start, length) or None;
    expert e's seg1 holds its first min(n_e, S1) tokens.
    """
    counts = np.asarray(counts)
    over = np.maximum(counts - S1, 0)
    for S2 in (64, 128, 192, 256, 384, 512):
        need = int(np.ceil(over / S2).sum())
        if need <= E:
            plist = []
            for e in range(E):
                o, st = int(over[e]), S1
                while o > 0:
                    ln = min(o, S2)
                    plist.append((e, st, ln))
                    st += ln
                    o -= ln
            pieces = [None] * E
            for i, pc in enumerate(plist):
                pieces[i] = pc
            return S2, pieces
    return None, None  # fall back to single big segment


def kernel(x, gating_output, w1_q, w2_q, w1_scale, w2_scale):
    global LAST_RESULTS
    from concourse.bass_utils import run_bass_kernel_spmd

    x = np.asarray(x, np.float32)
    w1_q = np.asarray(w1_q)
    w2_q = np.asarray(w2_q)
    w1_scale = np.asarray(w1_scale, np.float32)
    w2_scale = np.asarray(w2_scale, np.float32)

    topk_w, topk_ids = _route(gating_output)

    token_lists, coefs = [], []
    for e in range(E):
        mask = topk_ids == e
        tok = np.nonzero(mask.any(axis=1))[0]
        cf = np.where(mask, topk_w, 0.0).sum(axis=1)[tok].astype(np.float32)
        token_lists.append(tok)
        coefs.append(cf)
    counts = np.array([len(t) for t in token_lists])

    S1 = 1024
    S2, pieces = _pack(counts, S1)
    if S2 is None:
        S1 = max(P, int(-(-counts.max() // P)) * P)
        S2, pieces = 0, [None] * E

    key = (S1, S2)
    if key not in _BUILD_CACHE:
        nc = _build(S1, S2)
        nc.finalize()
        _BUILD_CACHE[key] = nc
    nc = _BUILD_CACHE[key]

    # host-side weight prep, once per expert
    w1h, w2h = [], []
    for e in range(E):
        w1d = _dequant_bf16(w1_q[e], w1_scale[e])   # [D, 2F]
        w1h.append(np.ascontiguousarray(
            w1d.reshape(KO1, P, NCH, 1024).transpose(1, 2, 0, 3)
        ))
        w2d = _dequant_bf16(w2_q[e], w2_scale[e])   # [F, D]
        w2h.append(np.ascontiguousarray(
            w2d.reshape(FH, P, DO, P).transpose(1, 2, 0, 3)
        ))

    def xT_of(tok, C):
        xe = np.zeros((C, D), np.float32)
        xe[: len(tok)] = x[tok]
        # [C, D] -> [P, KO1, C] with xT[p, k, t] = x[t, k*P + p]
        return np.ascontiguousarray(
            xe.T.reshape(KO1, P, C).transpose(1, 0, 2)
        ).astype(ml_dtypes.bfloat16)

    in_maps = []
    for e in range(E):
        tok1 = token_lists[e][: min(counts[e], S1)]
        m = {"xT1": xT_of(tok1, S1), "w1a": w1h[e], "w2a": w2h[e]}
        if S2:
            pc = pieces[e]
            if pc is None:
                m["xT2"] = np.zeros((P, KO1, S2), ml_dtypes.bfloat16)
                m["w1b"], m["w2b"] = w1h[e], w2h[e]
            else:
                pe, st, ln = pc
                m["xT2"] = xT_of(token_lists[pe][st : st + ln], S2)
                m["w1b"], m["w2b"] = w1h[pe], w2h[pe]
        in_maps.append(m)

    LAST_RESULTS = run_bass_kernel_spmd(nc, in_maps, core_ids=list(range(E)))

    out = np.zeros((T, D), np.float32)
    for e in range(E):
        res = LAST_RESULTS.results[e]
        y1 = res["yT1"].transpose(1, 0, 2).reshape(D, S1).T  # [S1, D]
        tok1 = token_lists[e][: min(counts[e], S1)]
        out[tok1] += coefs[e][: len(tok1), None] * y1[: len(tok1)]
        if S2 and pieces[e] is not None:
            pe, st, ln = pieces[e]
            y2 = res["yT2"].transpose(1, 0, 2).reshape(D, S2).T
            tok2 = token_lists[pe][st : st + ln]
            out[tok2] += coefs[pe][st : st + ln, None] * y2[:ln]
    return out


# revision 8
# speedup vs baseline: 1.0913x; 1.0913x over previous
"""GPTQ-Marlin sparse MoE layer for 8 Trainium2 NeuronCores.

Strategy (expert-parallel with overflow segments, host-side dispatch):
  - Router (softmax + top-2 + renormalize) replicated with the same jax ops
    as the reference so expert selection matches bit-for-bit.
  - Per-core uniform SPMD structure of two token segments:
      seg1: S1 (=1024) token slots running one "main" expert,
      seg2: S2 (=64)  token slots running an "overflow" expert.
    Expert e's first S1 tokens go to core e's seg1; tokens beyond S1 are
    split into <=S2-sized pieces placed on cores' seg2 slots (preferring the
    expert's own core so no second weight set is actually distinct). This
    cuts per-core token capacity from pad128(max_e n_e) (=1152 for the
    reference routing) to S1+S2 (=1088); the kernel is bf16 tensor-engine
    bound, so capacity is wall-clock.
  - GPTQ int4 codes are dequantized to bf16 on the host; each core streams
    its experts' W1 [D,2F] / W2 [F,D] from HBM.
  - Device kernel per core and segment: h = x @ W1 (transposed layout),
    act = silu(gate)*up, y = act @ W2 -- bf16 matmuls, fp32 PSUM.
  - Host applies the top-k coefficients during the scatter-add combine.

Layout (activations keep tokens on the free dim; no on-device transposes):
  mm1: psum[n, t] = sum_k W1[k, n] * xT[k, t]   (lhsT = W1 as stored)
  mm2: psum[d, t] = sum_f W2[f, d] * actT[f, t] (lhsT = W2 as stored)

Schedule notes (all measured on HW traces):
  - seg1/seg2 work is interleaved chunk-by-chunk (mm1) and d-tile by d-tile
    (mm2) so the ~48 MiB/core of weight DMA streams at a flat ~140 GB/s
    instead of starving the tiny seg2 at the end.
  - seg1 weights issue on the sync queue, seg2 weights + W2b on gpsimd,
    x / y on scalar: three independent in-order DMA queues, no blocking.
  - W1 chunks and x live in SBUF as [P, 2, KO1, 512] (column/token halves)
    written by separate DMAs, so the first matmul only needs ~3 MiB landed
    and j0..j3 can run a full contraction while the rest streams.
  - silu is applied in-place on the gate PSUM bank (scalar engine only ever
    runs Silu; a dummy activation at t~0 preloads its table), and the DVE
    does the up-multiply and the mm2 PSUM->SBUF copies.
  - y is written back as bf16 (host casts to f32): halves write traffic.
"""

import numpy as np
import ml_dtypes

E, T, D, F, TOPK, GROUP = 8, 4096, 1024, 4096, 2, 128
P = 128
KO1 = D // P           # 8  k-tiles for mm1
FH = F // P            # 32 act tiles (and k-tiles for mm2)
DO = D // P            # 8  output d-tiles
NCH = (2 * F) // 1024  # 8 column chunks of W1 (0..3 gate, 4..7 up)
TC = 512               # token chunk (one PSUM bank of fp32)

LAST_RESULTS = None    # test harness introspection

_BUILD_CACHE = {}


def _route(gating_output):
    """softmax + top-k + renormalize, replicated exactly like the reference."""
    try:
        import jax
        import jax.numpy as jnp

        scores = jax.nn.softmax(jnp.asarray(gating_output, jnp.float32), axis=-1)
        topk_w, topk_ids = jax.lax.top_k(scores, TOPK)
        topk_w = topk_w / jnp.sum(topk_w, axis=-1, keepdims=True)
        return np.asarray(topk_w, np.float32), np.asarray(topk_ids)
    except Exception:
        g = np.asarray(gating_output, np.float32)
        ex = np.exp(g - g.max(axis=-1, keepdims=True))
        s = (ex / ex.sum(axis=-1, keepdims=True)).astype(np.float32)
        ids = np.argsort(-s, axis=-1, kind="stable")[:, :TOPK]
        w = np.take_along_axis(s, ids, axis=-1)
        w = (w / w.sum(axis=-1, keepdims=True)).astype(np.float32)
        return w, ids


def _dequant_bf16(q, s):
    """q: [K, N] int codes, s: [K//GROUP, N] scales -> bf16 [K, N]."""
    w = (np.asarray(q, np.float32) - 8.0) * np.repeat(
        np.asarray(s, np.float32), GROUP, axis=0
    )
    return w.astype(ml_dtypes.bfloat16)


def _build(C1, C2, native_silu=True):
    """Per-core FFN program: seg1 of C1 tokens (expert A, C1 = n*512) +
    seg2 of C2 tokens (expert B, C2 <= 512, may be 0)."""
    import concourse.mybir as mybir
    import concourse.tile as tile
    from concourse import bacc

    assert C1 % TC == 0
    NT1 = C1 // TC
    assert 0 <= C2 <= TC

    nc = bacc.Bacc("TRN2", name="moe_expert_ffn")
    bf16 = mybir.dt.bfloat16
    f32 = mybir.dt.float32

    xT1 = nc.dram_tensor("xT1", [P, KO1, C1], bf16, kind="ExternalInput")
    w1a = nc.dram_tensor("w1a", [P, NCH, KO1, 1024], bf16, kind="ExternalInput")
    w2a = nc.dram_tensor("w2a", [P, DO, FH, P], bf16, kind="ExternalInput")
    yT1 = nc.dram_tensor("yT1", [P, DO, C1], bf16, kind="ExternalOutput")
    if C2:
        xT2 = nc.dram_tensor("xT2", [P, KO1, C2], bf16, kind="ExternalInput")
        w1b = nc.dram_tensor("w1b", [P, NCH, KO1, 1024], bf16, kind="ExternalInput")
        w2b = nc.dram_tensor("w2b", [P, DO, FH, P], bf16, kind="ExternalInput")
        yT2 = nc.dram_tensor("yT2", [P, DO, C2], bf16, kind="ExternalOutput")

    with tile.TileContext(nc) as tc:
        with (
            tc.tile_pool(name="xpool", bufs=1) as xpool,
            tc.tile_pool(name="w1pool", bufs=4) as w1pool,
            tc.tile_pool(name="w1bpool", bufs=2) as w1bpool,
            tc.tile_pool(name="w2pool", bufs=2) as w2pool,
            tc.tile_pool(name="actpool", bufs=1) as actpool,
            tc.tile_pool(name="miscpool", bufs=1) as miscpool,
            tc.tile_pool(name="ypool", bufs=1) as ypool,
            tc.tile_pool(name="pspool", bufs=6, space="PSUM") as pspool,
            tc.tile_pool(name="psypool", bufs=2, space="PSUM") as psypool,
        ):
            # x for seg1, stored as [P, token-half, k, 512] so each DMA piece
            # is a contiguous subtile
            x1 = xpool.tile([P, NT1, KO1, TC], bf16, tag="x1")
            for h in range(NT1):
                nc.scalar.dma_start(x1[:, h], xT1[:, :, h * TC : (h + 1) * TC])
            # preload the ACT engine's Silu table during the DMA wait
            warm_in = miscpool.tile([P, 8], f32, tag="warm")
            warm_out = miscpool.tile([P, 8], f32, tag="warm2")
            nc.gpsimd.memset(warm_in[:], 0.0)
            if native_silu:
                nc.scalar.activation(
                    warm_out[:], warm_in[:], mybir.ActivationFunctionType.Silu
                )
            if C2:
                x2 = xpool.tile([P, KO1, C2], bf16, tag="x2")
                nc.scalar.dma_start(x2[:], xT2[:])

            act1 = actpool.tile([P, FH, C1], bf16, tag="act1")
            if C2:
                act2 = actpool.tile([P, FH, C2], bf16, tag="act2")

            def mm1_chunk(c, wg, wu, nt, xt, act, Cw):
                """One W1 column-chunk (128*8 gate + up cols) over nt token
                chunks of width Cw (<=512)."""
                for h in range(nt):
                    xh = xt[:, h] if nt > 1 else xt
                    for j in range(8):
                        cg, jj = j // 4, j % 4
                        i = c * 8 + j
                        psg = pspool.tile([P, TC], f32, tag="ps")
                        psu = pspool.tile([P, TC], f32, tag="ps")
                        for k in range(KO1):
                            nc.tensor.matmul(
                                psg[:, :Cw],
                                wg[:, cg, k, jj * P : (jj + 1) * P],
                                xh[:, k, :Cw],
                                start=(k == 0),
                                stop=(k == KO1 - 1),
                            )
                            nc.tensor.matmul(
                                psu[:, :Cw],
                                wu[:, cg, k, jj * P : (jj + 1) * P],
                                xh[:, k, :Cw],
                                start=(k == 0),
                                stop=(k == KO1 - 1),
                            )
                        # silu(gate) -> bf16 SBUF staging (DVE may read at
                        # most one PSUM operand, so the mult needs sg in SB)
                        sg = miscpool.tile([P, TC], bf16, tag="sg", bufs=1)
                        if native_silu:
                            nc.scalar.activation(
                                sg[:, :Cw], psg[:, :Cw],
                                mybir.ActivationFunctionType.Silu,
                            )
                        else:
                            nc.scalar.activation(
                                sg[:, :Cw], psg[:, :Cw],
                                mybir.ActivationFunctionType.Sigmoid,
                            )
                            nc.vector.tensor_tensor(
                                sg[:, :Cw], sg[:, :Cw], psg[:, :Cw],
                                mybir.AluOpType.mult,
                            )
                        nc.vector.tensor_tensor(
                            act[:, i, h * TC : h * TC + Cw],
                            sg[:, :Cw],
                            psu[:, :Cw],
                            mybir.AluOpType.mult,
                        )

            # ---- mm1, seg1 chunk c then seg2 chunk c ----
            for c in range(NCH // 2):
                wg = w1pool.tile([P, 2, KO1, TC], bf16, tag="w1c")
                wu = w1pool.tile([P, 2, KO1, TC], bf16, tag="w1c")
                for cg in range(2):
                    nc.sync.dma_start(
                        wg[:, cg], w1a[:, c, :, cg * TC : (cg + 1) * TC]
                    )
                    nc.sync.dma_start(
                        wu[:, cg], w1a[:, c + 4, :, cg * TC : (cg + 1) * TC]
                    )
                mm1_chunk(c, wg, wu, NT1, x1, act1, TC)
                if C2:
                    wgb = w1bpool.tile([P, 2, KO1, TC], bf16, tag="w1b")
                    wub = w1bpool.tile([P, 2, KO1, TC], bf16, tag="w1b")
                    for cg in range(2):
                        nc.gpsimd.dma_start(
                            wgb[:, cg], w1b[:, c, :, cg * TC : (cg + 1) * TC]
                        )
                        nc.gpsimd.dma_start(
                            wub[:, cg], w1b[:, c + 4, :, cg * TC : (cg + 1) * TC]
                        )
                    mm1_chunk(c, wgb, wub, 1, x2, act2, C2)

            def mm2_d(d, ws, nt, act, yT, Cw):
                for h in range(nt):
                    psy = psypool.tile([P, TC], f32, tag="psy")
                    for k2 in range(FH):
                        nc.tensor.matmul(
                            psy[:, :Cw],
                            ws[:, k2],
                            act[:, k2, h * TC : h * TC + Cw],
                            start=(k2 == 0),
                            stop=(k2 == FH - 1),
                        )
                    yo = ypool.tile([P, TC], bf16, tag="yo")
                    nc.vector.tensor_copy(yo[:, :Cw], psy[:, :Cw])
                    nc.scalar.dma_start(
                        yT[:, d, h * TC : h * TC + Cw], yo[:, :Cw]
                    )

            # ---- mm2, seg1 d-tile then seg2 d-tile ----
            for d in range(DO):
                wsa = w2pool.tile([P, FH, P], bf16, tag="w2a", bufs=2)
                nc.sync.dma_start(wsa[:], w2a[:, d])
                mm2_d(d, wsa, NT1, act1, yT1, TC)
                if C2:
                    wsb = w2pool.tile([P, FH, P], bf16, tag="w2b", bufs=1)
                    nc.gpsimd.dma_start(wsb[:], w2b[:, d])
                    mm2_d(d, wsb, 1, act2, yT2, C2)
    return nc


def _pack(counts, S1=1024):
    """Choose seg2 size S2 and overflow piece placement.

    Returns (S2, pieces); pieces[core] = (expert, start, length) or None.
    Pieces are assigned to the overflowing expert's own core first (so its
    seg2 weight set is the same data), then to cores with free seg2 slots.
    """
    counts = np.asarray(counts)
    over = np.maximum(counts - S1, 0)
    for S2 in (64, 128, 192, 256, 384, 512):
        if int(np.ceil(over / S2).sum()) > E:
            continue
        pieces = [None] * E
        rest = []
        for e in range(E):
            o, st = int(over[e]), S1
            own = True
            while o > 0:
                ln = min(o, S2)
                if own and pieces[e] is None:
                    pieces[e] = (e, st, ln)
                    own = False
                else:
                    rest.append((e, st, ln))
                st += ln
                o -= ln
        free = [i for i in range(E) if pieces[i] is None]
        if len(rest) > len(free):
            continue
        for slot, pc in zip(free, rest):
            pieces[slot] = pc
        return S2, pieces
    return None, None


def kernel(x, gating_output, w1_q, w2_q, w1_scale, w2_scale):
    global LAST_RESULTS
    from concourse.bass_utils import run_bass_kernel_spmd

    x = np.asarray(x, np.float32)
    w1_q = np.asarray(w1_q)
    w2_q = np.asarray(w2_q)
    w1_scale = np.asarray(w1_scale, np.float32)
    w2_scale = np.asarray(w2_scale, np.float32)

    topk_w, topk_ids = _route(gating_output)

    token_lists, coefs = [], []
    for e in range(E):
        mask = topk_ids == e
        tok = np.nonzero(mask.any(axis=1))[0]
        cf = np.where(mask, topk_w, 0.0).sum(axis=1)[tok].astype(np.float32)
        token_lists.append(tok)
        coefs.append(cf)
    counts = np.array([len(t) for t in token_lists])

    S1 = 1024
    S2, pieces = _pack(counts, S1)
    if S2 is None:
        S1 = max(TC, int(-(-counts.max() // TC)) * TC)
        S2, pieces = 0, [None] * E

    key = (S1, S2)
    if key not in _BUILD_CACHE:
        nc = _build(S1, S2)
        nc.finalize()
        _BUILD_CACHE[key] = nc
    nc = _BUILD_CACHE[key]

    # host-side weight prep, once per expert
    w1h, w2h = [], []
    for e in range(E):
        w1d = _dequant_bf16(w1_q[e], w1_scale[e])   # [D, 2F]
        w1h.append(np.ascontiguousarray(
            w1d.reshape(KO1, P, NCH, 1024).transpose(1, 2, 0, 3)
        ))
        w2d = _dequant_bf16(w2_q[e], w2_scale[e])   # [F, D]
        w2h.append(np.ascontiguousarray(
            w2d.reshape(FH, P, DO, P).transpose(1, 2, 0, 3)
        ))

    def xT_of(tok, C):
        xe = np.zeros((C, D), np.float32)
        xe[: len(tok)] = x[tok]
        # [C, D] -> [P, KO1, C] with xT[p, k, t] = x[t, k*P + p]
        return np.ascontiguousarray(
            xe.T.reshape(KO1, P, C).transpose(1, 0, 2)
        ).astype(ml_dtypes.bfloat16)

    in_maps = []
    for e in range(E):
        tok1 = token_lists[e][: min(counts[e], S1)]
        m = {"xT1": xT_of(tok1, S1), "w1a": w1h[e], "w2a": w2h[e]}
        if S2:
            pc = pieces[e]
            if pc is None:
                m["xT2"] = np.zeros((P, KO1, S2), ml_dtypes.bfloat16)
                m["w1b"], m["w2b"] = w1h[e], w2h[e]
            else:
                pe, st, ln = pc
                m["xT2"] = xT_of(token_lists[pe][st : st + ln], S2)
                m["w1b"], m["w2b"] = w1h[pe], w2h[pe]
        in_maps.append(m)

    LAST_RESULTS = run_bass_kernel_spmd(nc, in_maps, core_ids=list(range(E)))

    out = np.zeros((T, D), np.float32)
    for e in range(E):
        res = LAST_RESULTS.results[e]
        y1 = np.asarray(res["yT1"], np.float32)
        y1 = y1.transpose(1, 0, 2).reshape(D, S1).T  # [S1, D]
        tok1 = token_lists[e][: min(counts[e], S1)]
        out[tok1] += coefs[e][: len(tok1), None] * y1[: len(tok1)]
        if S2 and pieces[e] is not None:
            pe, st, ln = pieces[e]
            y2 = np.asarray(res["yT2"], np.float32)
            y2 = y2.transpose(1, 0, 2).reshape(D, S2).T
            tok2 = token_lists[pe][st : st + ln]
            out[tok2] += coefs[pe][st : st + ln, None] * y2[:ln]
    return out


# revision 11
# speedup vs baseline: 1.1089x; 1.0162x over previous
"""GPTQ-Marlin sparse MoE layer for 8 Trainium2 NeuronCores.

Strategy (expert-parallel with overflow segments, host-side dispatch):
  - Router (softmax + top-2 + renormalize) replicated with the same jax ops
    as the reference so expert selection matches bit-for-bit.
  - Per-core uniform SPMD structure of two token segments:
      seg1: S1 (=1024) token slots running one "main" expert,
      seg2: S2 (=64)  token slots running an "overflow" expert.
    Expert e's first S1 tokens go to core e's seg1; tokens beyond S1 are
    split into <=S2-sized pieces placed on cores' seg2 slots (preferring the
    expert's own core so no second weight set is actually distinct). This
    cuts per-core token capacity from pad128(max_e n_e) (=1152 for the
    reference routing) to S1+S2 (=1088); the kernel is bf16 tensor-engine
    bound, so capacity is wall-clock.
  - GPTQ int4 codes are dequantized to bf16 on the host; each core streams
    its experts' W1 [D,2F] / W2 [F,D] from HBM.
  - Device kernel per core and segment: h = x @ W1 (transposed layout),
    act = silu(gate)*up, y = act @ W2 -- bf16 matmuls, fp32 PSUM.
  - Host applies the top-k coefficients during the scatter-add combine.

Layout (activations keep tokens on the free dim; no on-device transposes):
  mm1: psum[n, t] = sum_k W1[k, n] * xT[k, t]   (lhsT = W1 as stored)
  mm2: psum[d, t] = sum_f W2[f, d] * actT[f, t] (lhsT = W2 as stored)

Schedule notes (all measured on HW traces):
  - seg1/seg2 work is interleaved chunk-by-chunk (mm1) and d-tile by d-tile
    (mm2) so the ~48 MiB/core of weight DMA streams at a flat ~140 GB/s
    instead of starving the tiny seg2 at the end.
  - seg1 weights issue on the sync queue, seg2 weights + W2b on gpsimd,
    x / y on scalar: three independent in-order DMA queues, no blocking.
  - W1 chunks and x live in SBUF as [P, 2, KO1, 512] (column/token halves)
    written by separate DMAs, so the first matmul only needs ~3 MiB landed
    and j0..j3 can run a full contraction while the rest streams.
  - silu is applied in-place on the gate PSUM bank (scalar engine only ever
    runs Silu; a dummy activation at t~0 preloads its table), and the DVE
    does the up-multiply and the mm2 PSUM->SBUF copies.
  - y is written back as bf16 (host casts to f32): halves write traffic.
"""

import numpy as np
import ml_dtypes

E, T, D, F, TOPK, GROUP = 8, 4096, 1024, 4096, 2, 128
P = 128
KO1 = D // P           # 8  k-tiles for mm1
FH = F // P            # 32 act tiles (and k-tiles for mm2)
DO = D // P            # 8  output d-tiles
NCH = (2 * F) // 1024  # 8 column chunks of W1 (0..3 gate, 4..7 up)
TC = 512               # token chunk (one PSUM bank of fp32)

LAST_RESULTS = None    # test harness introspection

_BUILD_CACHE = {}


def _route(gating_output):
    """softmax + top-k + renormalize, replicated exactly like the reference."""
    try:
        import jax
        import jax.numpy as jnp

        scores = jax.nn.softmax(jnp.asarray(gating_output, jnp.float32), axis=-1)
        topk_w, topk_ids = jax.lax.top_k(scores, TOPK)
        topk_w = topk_w / jnp.sum(topk_w, axis=-1, keepdims=True)
        return np.asarray(topk_w, np.float32), np.asarray(topk_ids)
    except Exception:
        g = np.asarray(gating_output, np.float32)
        ex = np.exp(g - g.max(axis=-1, keepdims=True))
        s = (ex / ex.sum(axis=-1, keepdims=True)).astype(np.float32)
        ids = np.argsort(-s, axis=-1, kind="stable")[:, :TOPK]
        w = np.take_along_axis(s, ids, axis=-1)
        w = (w / w.sum(axis=-1, keepdims=True)).astype(np.float32)
        return w, ids


def _dequant_bf16(q, s):
    """q: [K, N] int codes, s: [K//GROUP, N] scales -> bf16 [K, N]."""
    w = (np.asarray(q, np.float32) - 8.0) * np.repeat(
        np.asarray(s, np.float32), GROUP, axis=0
    )
    return w.astype(ml_dtypes.bfloat16)


def _build(C1, C2, native_silu=True):
    """Per-core FFN program: seg1 of C1 tokens (expert A, C1 = n*512) +
    seg2 of C2 tokens (expert B, C2 <= 512, may be 0)."""
    import concourse.mybir as mybir
    import concourse.tile as tile
    from concourse import bacc

    assert C1 % TC == 0
    NT1 = C1 // TC
    assert 0 <= C2 <= TC

    nc = bacc.Bacc("TRN2", name="moe_expert_ffn")
    bf16 = mybir.dt.bfloat16
    f32 = mybir.dt.float32

    xT1 = nc.dram_tensor("xT1", [P, KO1, C1], bf16, kind="ExternalInput")
    w1a = nc.dram_tensor("w1a", [P, NCH, KO1, 1024], bf16, kind="ExternalInput")
    w2a = nc.dram_tensor("w2a", [P, DO, FH, P], bf16, kind="ExternalInput")
    yT1 = nc.dram_tensor("yT1", [P, DO, C1], bf16, kind="ExternalOutput")
    if C2:
        xT2 = nc.dram_tensor("xT2", [P, KO1, C2], bf16, kind="ExternalInput")
        w1b = nc.dram_tensor("w1b", [P, NCH, KO1, 1024], bf16, kind="ExternalInput")
        w2b = nc.dram_tensor("w2b", [P, DO, FH, P], bf16, kind="ExternalInput")
        yT2 = nc.dram_tensor("yT2", [P, DO, C2], bf16, kind="ExternalOutput")

    with tile.TileContext(nc) as tc:
        with (
            tc.tile_pool(name="xpool", bufs=1) as xpool,
            tc.tile_pool(name="w1pool", bufs=4) as w1pool,
            tc.tile_pool(name="w1bpool", bufs=2) as w1bpool,
            tc.tile_pool(name="w2pool", bufs=2) as w2pool,
            tc.tile_pool(name="actpool", bufs=1) as actpool,
            tc.tile_pool(name="miscpool", bufs=1) as miscpool,
            tc.tile_pool(name="ypool", bufs=1) as ypool,
            tc.tile_pool(name="pspool", bufs=6, space="PSUM") as pspool,
            tc.tile_pool(name="psypool", bufs=2, space="PSUM") as psypool,
        ):
            # x for seg1, stored as [P, token-half, k, 512] so each DMA piece
            # is a contiguous subtile
            x1 = xpool.tile([P, NT1, KO1, TC], bf16, tag="x1")
            for h in range(NT1):
                nc.scalar.dma_start(x1[:, h], xT1[:, :, h * TC : (h + 1) * TC])
            # preload the ACT engine's Silu table during the DMA wait
            warm_in = miscpool.tile([P, 8], f32, tag="warm")
            warm_out = miscpool.tile([P, 8], f32, tag="warm2")
            nc.gpsimd.memset(warm_in[:], 0.0)
            if native_silu:
                nc.scalar.activation(
                    warm_out[:], warm_in[:], mybir.ActivationFunctionType.Silu
                )
            if C2:
                x2 = xpool.tile([P, KO1, C2], bf16, tag="x2")
                nc.scalar.dma_start(x2[:], xT2[:])

            act1 = actpool.tile([P, FH, C1], bf16, tag="act1")
            if C2:
                act2 = actpool.tile([P, FH, C2], bf16, tag="act2")

            def mm1_chunk(c, wg, wu, nt, xt, act, Cw):
                """One W1 column-chunk (128*8 gate + up cols) over nt token
                chunks of width Cw (<=512)."""
                for h in range(nt):
                    xh = xt[:, h] if nt > 1 else xt
                    for j in range(8):
                        cg, jj = j // 4, j % 4
                        i = c * 8 + j
                        psg = pspool.tile([P, TC], f32, tag="ps")
                        psu = pspool.tile([P, TC], f32, tag="ps")
                        for k in range(KO1):
                            nc.tensor.matmul(
                                psg[:, :Cw],
                                wg[:, cg, k, jj * P : (jj + 1) * P],
                                xh[:, k, :Cw],
                                start=(k == 0),
                                stop=(k == KO1 - 1),
                            )
                            nc.tensor.matmul(
                                psu[:, :Cw],
                                wu[:, cg, k, jj * P : (jj + 1) * P],
                                xh[:, k, :Cw],
                                start=(k == 0),
                                stop=(k == KO1 - 1),
                            )
                        # silu(gate) -> bf16 SBUF staging (DVE may read at
                        # most one PSUM operand, so the mult needs sg in SB)
                        sg = miscpool.tile([P, TC], bf16, tag="sg", bufs=1)
                        if native_silu:
                            nc.scalar.activation(
                                sg[:, :Cw], psg[:, :Cw],
                                mybir.ActivationFunctionType.Silu,
                            )
                        else:
                            nc.scalar.activation(
                                sg[:, :Cw], psg[:, :Cw],
                                mybir.ActivationFunctionType.Sigmoid,
                            )
                            nc.vector.tensor_tensor(
                                sg[:, :Cw], sg[:, :Cw], psg[:, :Cw],
                                mybir.AluOpType.mult,
                            )
                        nc.vector.tensor_tensor(
                            act[:, i, h * TC : h * TC + Cw],
                            sg[:, :Cw],
                            psu[:, :Cw],
                            mybir.AluOpType.mult,
                        )

            # ---- mm1, seg1 chunk c then seg2 chunk c ----
            for c in range(NCH // 2):
                wg = w1pool.tile([P, 2, KO1, TC], bf16, tag="w1c")
                wu = w1pool.tile([P, 2, KO1, TC], bf16, tag="w1c")
                # chunk 0 is on the startup critical path: give gate-weights
                # to the sync queue and up-weights to gpsimd so the first
                # j-block only waits ~1 MiB per queue
                up_eng = nc.gpsimd if c == 0 else nc.sync
                for cg in range(2):
                    nc.sync.dma_start(
                        wg[:, cg], w1a[:, c, :, cg * TC : (cg + 1) * TC]
                    )
                    up_eng.dma_start(
                        wu[:, cg], w1a[:, c + 4, :, cg * TC : (cg + 1) * TC]
                    )
                mm1_chunk(c, wg, wu, NT1, x1, act1, TC)
                if C2:
                    wgb = w1bpool.tile([P, 2, KO1, TC], bf16, tag="w1b")
                    wub = w1bpool.tile([P, 2, KO1, TC], bf16, tag="w1b")
                    for cg in range(2):
                        nc.gpsimd.dma_start(
                            wgb[:, cg], w1b[:, c, :, cg * TC : (cg + 1) * TC]
                        )
                        nc.gpsimd.dma_start(
                            wub[:, cg], w1b[:, c + 4, :, cg * TC : (cg + 1) * TC]
                        )
                    mm1_chunk(c, wgb, wub, 1, x2, act2, C2)

            def mm2_d(d, ws, nt, act, yT, Cw, out_eng):
                for h in range(nt):
                    psy = psypool.tile([P, TC], f32, tag="psy")
                    for k2 in range(FH):
                        nc.tensor.matmul(
                            psy[:, :Cw],
                            ws[:, k2],
                            act[:, k2, h * TC : h * TC + Cw],
                            start=(k2 == 0),
                            stop=(k2 == FH - 1),
                        )
                    yo = ypool.tile([P, TC], bf16, tag="yo")
                    nc.vector.tensor_copy(yo[:, :Cw], psy[:, :Cw])
                    out_eng.dma_start(
                        yT[:, d, h * TC : h * TC + Cw], yo[:, :Cw]
                    )

            # ---- mm2, seg1 d-tile then seg2 d-tile ----
            for d in range(DO):
                wsa = w2pool.tile([P, FH, P], bf16, tag="w2a", bufs=2)
                nc.sync.dma_start(wsa[:], w2a[:, d])
                mm2_d(d, wsa, NT1, act1, yT1, TC, nc.sync)
                if C2:
                    wsb = w2pool.tile([P, FH, P], bf16, tag="w2b", bufs=1)
                    nc.gpsimd.dma_start(wsb[:], w2b[:, d])
                    mm2_d(d, wsb, 1, act2, yT2, C2, nc.scalar)
    return nc


def _pack(counts, S1=1024):
    """Choose seg2 size S2 and overflow piece placement.

    Returns (S2, pieces); pieces[core] = (expert, start, length) or None.
    Pieces are assigned to the overflowing expert's own core first (so its
    seg2 weight set is the same data), then to cores with free seg2 slots.
    """
    counts = np.asarray(counts)
    over = np.maximum(counts - S1, 0)
    for S2 in (64, 128, 192, 256, 384, 512):
        if int(np.ceil(over / S2).sum()) > E:
            continue
        pieces = [None] * E
        rest = []
        for e in range(E):
            o, st = int(over[e]), S1
            own = True
            while o > 0:
                ln = min(o, S2)
                if own and pieces[e] is None:
                    pieces[e] = (e, st, ln)
                    own = False
                else:
                    rest.append((e, st, ln))
                st += ln
                o -= ln
        free = [i for i in range(E) if pieces[i] is None]
        if len(rest) > len(free):
            continue
        for slot, pc in zip(free, rest):
            pieces[slot] = pc
        return S2, pieces
    return None, None


def kernel(x, gating_output, w1_q, w2_q, w1_scale, w2_scale):
    global LAST_RESULTS
    from concourse.bass_utils import run_bass_kernel_spmd

    x = np.asarray(x, np.float32)
    w1_q = np.asarray(w1_q)
    w2_q = np.asarray(w2_q)
    w1_scale = np.asarray(w1_scale, np.float32)
    w2_scale = np.asarray(w2_scale, np.float32)

    topk_w, topk_ids = _route(gating_output)

    token_lists, coefs = [], []
    for e in range(E):
        mask = topk_ids == e
        tok = np.nonzero(mask.any(axis=1))[0]
        cf = np.where(mask, topk_w, 0.0).sum(axis=1)[tok].astype(np.float32)
        token_lists.append(tok)
        coefs.append(cf)
    counts = np.array([len(t) for t in token_lists])

    S1 = 1024
    S2, pieces = _pack(counts, S1)
    if S2 is None:
        S1 = max(TC, int(-(-counts.max() // TC)) * TC)
        S2, pieces = 0, [None] * E

    key = (S1, S2)
    if key not in _BUILD_CACHE:
        nc = _build(S1, S2)
        nc.finalize()
        _BUILD_CACHE[key] = nc
    nc = _BUILD_CACHE[key]

    # host-side weight prep, once per expert
    w1h, w2h = [], []
    for e in range(E):
        w1d = _dequant_bf16(w1_q[e], w1_scale[e])   # [D, 2F]
        w1h.append(np.ascontiguousarray(
            w1d.reshape(KO1, P, NCH, 1024).transpose(1, 2, 0, 3)
        ))
        w2d = _dequant_bf16(w2_q[e], w2_scale[e])   # [F, D]
        w2h.append(np.ascontiguousarray(
            w2d.reshape(FH, P, DO, P).transpose(1, 2, 0, 3)
        ))

    def xT_of(tok, C):
        xe = np.zeros((C, D), np.float32)
        xe[: len(tok)] = x[tok]
        # [C, D] -> [P, KO1, C] with xT[p, k, t] = x[t, k*P + p]
        return np.ascontiguousarray(
            xe.T.reshape(KO1, P, C).transpose(1, 0, 2)
        ).astype(ml_dtypes.bfloat16)

    in_maps = []
    for e in range(E):
        tok1 = token_lists[e][: min(counts[e], S1)]
        m = {"xT1": xT_of(tok1, S1), "w1a": w1h[e], "w2a": w2h[e]}
        if S2:
            pc = pieces[e]
            if pc is None:
                m["xT2"] = np.zeros((P, KO1, S2), ml_dtypes.bfloat16)
                m["w1b"], m["w2b"] = w1h[e], w2h[e]
            else:
                pe, st, ln = pc
                m["xT2"] = xT_of(token_lists[pe][st : st + ln], S2)
                m["w1b"], m["w2b"] = w1h[pe], w2h[pe]
        in_maps.append(m)

    LAST_RESULTS = run_bass_kernel_spmd(nc, in_maps, core_ids=list(range(E)))

    out = np.zeros((T, D), np.float32)
    for e in range(E):
        res = LAST_RESULTS.results[e]
        y1 = np.asarray(res["yT1"], np.float32)
        y1 = y1.transpose(1, 0, 2).reshape(D, S1).T  # [S1, D]
        tok1 = token_lists[e][: min(counts[e], S1)]
        out[tok1] += coefs[e][: len(tok1), None] * y1[: len(tok1)]
        if S2 and pieces[e] is not None:
            pe, st, ln = pieces[e]
            y2 = np.asarray(res["yT2"], np.float32)
            y2 = y2.transpose(1, 0, 2).reshape(D, S2).T
            tok2 = token_lists[pe][st : st + ln]
            out[tok2] += coefs[pe][st : st + ln, None] * y2[:ln]
    return out


# revision 15
# speedup vs baseline: 1.1197x; 1.0097x over previous
"""GPTQ-Marlin sparse MoE layer for 8 Trainium2 NeuronCores.

Strategy (expert-parallel with overflow segments, host-side dispatch):
  - Router (softmax + top-2 + renormalize) replicated with the same jax ops
    as the reference so expert selection matches bit-for-bit.
  - Per-core uniform SPMD structure of two token segments:
      seg1: S1 (=1024) token slots running one "main" expert,
      seg2: S2 (=64)  token slots running an "overflow" expert.
    Expert e's first S1 tokens go to core e's seg1; tokens beyond S1 are
    split into <=S2-sized pieces placed on cores' seg2 slots (preferring the
    expert's own core so no second weight set is actually distinct). This
    cuts per-core token capacity from pad128(max_e n_e) (=1152 for the
    reference routing) to S1+S2 (=1088); the kernel is bf16 tensor-engine
    bound, so capacity is wall-clock.
  - GPTQ int4 codes are dequantized to bf16 on the host; each core streams
    its experts' W1 [D,2F] / W2 [F,D] from HBM.
  - Device kernel per core and segment: h = x @ W1 (transposed layout),
    act = silu(gate)*up, y = act @ W2 -- bf16 matmuls, fp32 PSUM.
  - Host applies the top-k coefficients during the scatter-add combine.

Layout (activations keep tokens on the free dim; no on-device transposes):
  mm1: psum[n, t] = sum_k W1[k, n] * xT[k, t]   (lhsT = W1 as stored)
  mm2: psum[d, t] = sum_f W2[f, d] * actT[f, t] (lhsT = W2 as stored)

Schedule notes (all measured on HW traces):
  - seg1/seg2 work is interleaved chunk-by-chunk (mm1) and d-tile by d-tile
    (mm2) so the ~48 MiB/core of weight DMA streams at a flat ~140 GB/s
    instead of starving the tiny seg2 at the end.
  - seg1 weights issue on the sync queue, seg2 weights + W2b on gpsimd,
    x / y on scalar: three independent in-order DMA queues, no blocking.
  - W1 chunks and x live in SBUF as [P, 2, KO1, 512] (column/token halves)
    written by separate DMAs, so the first matmul only needs ~3 MiB landed
    and j0..j3 can run a full contraction while the rest streams.
  - silu is applied in-place on the gate PSUM bank (scalar engine only ever
    runs Silu; a dummy activation at t~0 preloads its table), and the DVE
    does the up-multiply and the mm2 PSUM->SBUF copies.
  - y is written back as bf16 (host casts to f32): halves write traffic.
"""

import numpy as np
import ml_dtypes

E, T, D, F, TOPK, GROUP = 8, 4096, 1024, 4096, 2, 128
P = 128
KO1 = D // P           # 8  k-tiles for mm1
FH = F // P            # 32 act tiles (and k-tiles for mm2)
DO = D // P            # 8  output d-tiles
NCH = (2 * F) // 1024  # 8 column chunks of W1 (0..3 gate, 4..7 up)
TC = 512               # token chunk (one PSUM bank of fp32)

LAST_RESULTS = None    # test harness introspection

_BUILD_CACHE = {}


def _route(gating_output):
    """softmax + top-k + renormalize, replicated exactly like the reference."""
    try:
        import jax
        import jax.numpy as jnp

        scores = jax.nn.softmax(jnp.asarray(gating_output, jnp.float32), axis=-1)
        topk_w, topk_ids = jax.lax.top_k(scores, TOPK)
        topk_w = topk_w / jnp.sum(topk_w, axis=-1, keepdims=True)
        return np.asarray(topk_w, np.float32), np.asarray(topk_ids)
    except Exception:
        g = np.asarray(gating_output, np.float32)
        ex = np.exp(g - g.max(axis=-1, keepdims=True))
        s = (ex / ex.sum(axis=-1, keepdims=True)).astype(np.float32)
        ids = np.argsort(-s, axis=-1, kind="stable")[:, :TOPK]
        w = np.take_along_axis(s, ids, axis=-1)
        w = (w / w.sum(axis=-1, keepdims=True)).astype(np.float32)
        return w, ids


def _dequant_bf16(q, s):
    """q: [K, N] int codes, s: [K//GROUP, N] scales -> bf16 [K, N]."""
    w = (np.asarray(q, np.float32) - 8.0) * np.repeat(
        np.asarray(s, np.float32), GROUP, axis=0
    )
    return w.astype(ml_dtypes.bfloat16)


def _build(C1, C2, native_silu=True):
    """Per-core FFN program: seg1 of C1 tokens (expert A, C1 = n*512) +
    seg2 of C2 tokens (expert B, C2 <= 512, may be 0)."""
    import concourse.mybir as mybir
    import concourse.tile as tile
    from concourse import bacc

    assert C1 % TC == 0
    NT1 = C1 // TC
    assert 0 <= C2 <= TC

    nc = bacc.Bacc("TRN2", name="moe_expert_ffn")
    bf16 = mybir.dt.bfloat16
    f32 = mybir.dt.float32

    xT1 = nc.dram_tensor("xT1", [P, KO1, C1], bf16, kind="ExternalInput")
    w1a = nc.dram_tensor("w1a", [P, NCH, KO1, 1024], bf16, kind="ExternalInput")
    w2a = nc.dram_tensor("w2a", [P, DO, FH, P], bf16, kind="ExternalInput")
    yT1 = nc.dram_tensor("yT1", [P, DO, C1], bf16, kind="ExternalOutput")
    if C2:
        xT2 = nc.dram_tensor("xT2", [P, KO1, C2], bf16, kind="ExternalInput")
        w1b = nc.dram_tensor("w1b", [P, NCH, KO1, 1024], bf16, kind="ExternalInput")
        w2b = nc.dram_tensor("w2b", [P, DO, FH, P], bf16, kind="ExternalInput")
        yT2 = nc.dram_tensor("yT2", [P, DO, C2], bf16, kind="ExternalOutput")

    with tile.TileContext(nc) as tc:
        with (
            tc.tile_pool(name="xpool", bufs=1) as xpool,
            tc.tile_pool(name="w1pool", bufs=4) as w1pool,
            tc.tile_pool(name="w1bpool", bufs=2) as w1bpool,
            tc.tile_pool(name="w2pool", bufs=2) as w2pool,
            tc.tile_pool(name="actpool", bufs=1) as actpool,
            tc.tile_pool(name="miscpool", bufs=1) as miscpool,
            tc.tile_pool(name="ypool", bufs=1) as ypool,
            tc.tile_pool(name="pspool", bufs=6, space="PSUM") as pspool,
            tc.tile_pool(name="psypool", bufs=2, space="PSUM") as psypool,
        ):
            # x for seg1: one tile per 512-token half (deps are tile-granular,
            # so separate tiles let the first matmul start after ~1 MiB)
            x1 = []
            for h in range(NT1):
                x1h = xpool.tile([P, KO1, TC], bf16, tag="x1", bufs=NT1,
                                 name=f"x1h{h}")
                nc.scalar.dma_start(x1h[:], xT1[:, :, h * TC : (h + 1) * TC])
                x1.append(x1h)
            # preload the ACT engine's Silu table during the DMA wait
            warm_in = miscpool.tile([P, 8], f32, tag="warm")
            warm_out = miscpool.tile([P, 8], f32, tag="warm2")
            nc.gpsimd.memset(warm_in[:], 0.0)
            if native_silu:
                nc.scalar.activation(
                    warm_out[:], warm_in[:], mybir.ActivationFunctionType.Silu
                )
            if C2:
                x2 = xpool.tile([P, KO1, C2], bf16, tag="x2")
                nc.scalar.dma_start(x2[:], xT2[:])

            act1 = actpool.tile([P, FH, C1], bf16, tag="act1")
            if C2:
                act2 = actpool.tile([P, FH, C2], bf16, tag="act2")

            def mm1_chunk(c, wg, wu, xts, act, Cw):
                """One W1 column-chunk (128*8 gate + up cols) over the token
                chunks in xts, each of width Cw (<=512). wg/wu are lists of
                two [P, KO1, TC] tiles (column-group halves)."""
                for h, xh in enumerate(xts):
                    for j in range(8):
                        cg, jj = j // 4, j % 4
                        i = c * 8 + j
                        psg = pspool.tile([P, TC], f32, tag="ps")
                        psu = pspool.tile([P, TC], f32, tag="ps")
                        for k in range(KO1):
                            nc.tensor.matmul(
                                psg[:, :Cw],
                                wg[cg][:, k, jj * P : (jj + 1) * P],
                                xh[:, k, :Cw],
                                start=(k == 0),
                                stop=(k == KO1 - 1),
                            )
                            nc.tensor.matmul(
                                psu[:, :Cw],
                                wu[cg][:, k, jj * P : (jj + 1) * P],
                                xh[:, k, :Cw],
                                start=(k == 0),
                                stop=(k == KO1 - 1),
                            )
                        # silu(gate) -> bf16 SBUF staging (DVE may read at
                        # most one PSUM operand, so the mult needs sg in SB)
                        sg = miscpool.tile([P, TC], bf16, tag="sg", bufs=1)
                        if native_silu:
                            nc.scalar.activation(
                                sg[:, :Cw], psg[:, :Cw],
                                mybir.ActivationFunctionType.Silu,
                            )
                        else:
                            nc.scalar.activation(
                                sg[:, :Cw], psg[:, :Cw],
                                mybir.ActivationFunctionType.Sigmoid,
                            )
                            nc.vector.tensor_tensor(
                                sg[:, :Cw], sg[:, :Cw], psg[:, :Cw],
                                mybir.AluOpType.mult,
                            )
                        nc.vector.tensor_tensor(
                            act[:, i, h * TC : h * TC + Cw],
                            sg[:, :Cw],
                            psu[:, :Cw],
                            mybir.AluOpType.mult,
                        )

            # ---- mm1, seg1 chunk c then seg2 chunk c ----
            for c in range(NCH // 2):
                wg, wu = [], []
                # chunk 0 is on the startup critical path: give gate-weights
                # to the sync queue and up-weights to gpsimd so the first
                # j-block only waits ~1 MiB per queue
                up_eng = nc.gpsimd if c == 0 else nc.sync
                for cg in range(2):
                    wgt = w1pool.tile([P, KO1, TC], bf16, tag="w1c", bufs=8,
                                      name=f"wg{c}{cg}")
                    nc.sync.dma_start(
                        wgt[:], w1a[:, c, :, cg * TC : (cg + 1) * TC]
                    )
                    wg.append(wgt)
                    wut = w1pool.tile([P, KO1, TC], bf16, tag="w1c", bufs=8,
                                      name=f"wu{c}{cg}")
                    up_eng.dma_start(
                        wut[:], w1a[:, c + 4, :, cg * TC : (cg + 1) * TC]
                    )
                    wu.append(wut)
                mm1_chunk(c, wg, wu, x1, act1, TC)
                if C2:
                    wgb, wub = [], []
                    for cg in range(2):
                        wgbt = w1bpool.tile([P, KO1, TC], bf16, tag="w1b",
                                            bufs=4, name=f"wgb{c}{cg}")
                        nc.gpsimd.dma_start(
                            wgbt[:], w1b[:, c, :, cg * TC : (cg + 1) * TC]
                        )
                        wgb.append(wgbt)
                        wubt = w1bpool.tile([P, KO1, TC], bf16, tag="w1b",
                                            bufs=4, name=f"wub{c}{cg}")
                        nc.gpsimd.dma_start(
                            wubt[:], w1b[:, c + 4, :, cg * TC : (cg + 1) * TC]
                        )
                        wub.append(wubt)
                    mm1_chunk(c, wgb, wub, [x2], act2, C2)

            FH2 = FH // 2

            def mm2_d(d, ws, nt, act, yT, Cw, out_engs):
                for h in range(nt):
                    psy = psypool.tile([P, TC], f32, tag="psy")
                    for k2 in range(FH):
                        nc.tensor.matmul(
                            psy[:, :Cw],
                            ws[k2 // FH2][:, k2 % FH2],
                            act[:, k2, h * TC : h * TC + Cw],
                            start=(k2 == 0),
                            stop=(k2 == FH - 1),
                        )
                    yo = ypool.tile([P, TC], bf16, tag="yo")
                    nc.vector.tensor_copy(yo[:, :Cw], psy[:, :Cw])
                    out_engs[h % len(out_engs)].dma_start(
                        yT[:, d, h * TC : h * TC + Cw], yo[:, :Cw]
                    )

            # ---- mm2, seg1 d-tile then seg2 d-tile ----
            for d in range(DO):
                wsa = []
                for g in range(2):
                    wsat = w2pool.tile([P, FH2, P], bf16, tag="w2a", bufs=4,
                                       name=f"wsa{d}{g}")
                    nc.sync.dma_start(wsat[:], w2a[:, d, g * FH2 : (g + 1) * FH2])
                    wsa.append(wsat)
                mm2_d(d, wsa, NT1, act1, yT1, TC, [nc.sync, nc.scalar])
                if C2:
                    wsb = []
                    for g in range(2):
                        wsbt = w2pool.tile([P, FH2, P], bf16, tag="w2b", bufs=2,
                                           name=f"wsb{d}{g}")
                        nc.gpsimd.dma_start(
                            wsbt[:], w2b[:, d, g * FH2 : (g + 1) * FH2]
                        )
                        wsb.append(wsbt)
                    mm2_d(d, wsb, 1, act2, yT2, C2, [nc.scalar])
    return nc


def _pack(counts, S1=1024):
    """Choose seg2 size S2 and overflow piece placement.

    Returns (S2, pieces); pieces[core] = (expert, start, length) or None.
    Pieces are assigned to the overflowing expert's own core first (so its
    seg2 weight set is the same data), then to cores with free seg2 slots.
    """
    counts = np.asarray(counts)
    over = np.maximum(counts - S1, 0)
    for S2 in (64, 128, 192, 256, 384, 512):
        if int(np.ceil(over / S2).sum()) > E:
            continue
        pieces = [None] * E
        rest = []
        for e in range(E):
            o, st = int(over[e]), S1
            own = True
            while o > 0:
                ln = min(o, S2)
                if own and pieces[e] is None:
                    pieces[e] = (e, st, ln)
                    own = False
                else:
                    rest.append((e, st, ln))
                st += ln
                o -= ln
        free = [i for i in range(E) if pieces[i] is None]
        if len(rest) > len(free):
            continue
        for slot, pc in zip(free, rest):
            pieces[slot] = pc
        return S2, pieces
    return None, None


def kernel(x, gating_output, w1_q, w2_q, w1_scale, w2_scale):
    global LAST_RESULTS
    from concourse.bass_utils import run_bass_kernel_spmd

    x = np.asarray(x, np.float32)
    w1_q = np.asarray(w1_q)
    w2_q = np.asarray(w2_q)
    w1_scale = np.asarray(w1_scale, np.float32)
    w2_scale = np.asarray(w2_scale, np.float32)

    topk_w, topk_ids = _route(gating_output)

    token_lists, coefs = [], []
    for e in range(E):
        mask = topk_ids == e
        tok = np.nonzero(mask.any(axis=1))[0]
        cf = np.where(mask, topk_w, 0.0).sum(axis=1)[tok].astype(np.float32)
        token_lists.append(tok)
        coefs.append(cf)
    counts = np.array([len(t) for t in token_lists])

    S1 = 1024
    S2, pieces = _pack(counts, S1)
    if S2 is None:
        S1 = max(TC, int(-(-counts.max() // TC)) * TC)
        S2, pieces = 0, [None] * E

    key = (S1, S2)
    if key not in _BUILD_CACHE:
        nc = _build(S1, S2)
        nc.finalize()
        _BUILD_CACHE[key] = nc
    nc = _BUILD_CACHE[key]

    # host-side weight prep, once per expert
    w1h, w2h = [], []
    for e in range(E):
        w1d = _dequant_bf16(w1_q[e], w1_scale[e])   # [D, 2F]
        w1h.append(np.ascontiguousarray(
            w1d.reshape(KO1, P, NCH, 1024).transpose(1, 2, 0, 3)
        ))
        w2d = _dequant_bf16(w2_q[e], w2_scale[e])   # [F, D]
        w2h.append(np.ascontiguousarray(
            w2d.reshape(FH, P, DO, P).transpose(1, 2, 0, 3)
        ))

    def xT_of(tok, C):
        xe = np.zeros((C, D), np.float32)
        xe[: len(tok)] = x[tok]
        # [C, D] -> [P, KO1, C] with xT[p, k, t] = x[t, k*P + p]
        return np.ascontiguousarray(
            xe.T.reshape(KO1, P, C).transpose(1, 0, 2)
        ).astype(ml_dtypes.bfloat16)

    in_maps = []
    for e in range(E):
        tok1 = token_lists[e][: min(counts[e], S1)]
        m = {"xT1": xT_of(tok1, S1), "w1a": w1h[e], "w2a": w2h[e]}
        if S2:
            pc = pieces[e]
            if pc is None:
                m["xT2"] = np.zeros((P, KO1, S2), ml_dtypes.bfloat16)
                m["w1b"], m["w2b"] = w1h[e], w2h[e]
            else:
                pe, st, ln = pc
                m["xT2"] = xT_of(token_lists[pe][st : st + ln], S2)
                m["w1b"], m["w2b"] = w1h[pe], w2h[pe]
        in_maps.append(m)

    LAST_RESULTS = run_bass_kernel_spmd(nc, in_maps, core_ids=list(range(E)))

    out = np.zeros((T, D), np.float32)
    for e in range(E):
        res = LAST_RESULTS.results[e]
        y1 = np.asarray(res["yT1"], np.float32)
        y1 = y1.transpose(1, 0, 2).reshape(D, S1).T  # [S1, D]
        tok1 = token_lists[e][: min(counts[e], S1)]
        out[tok1] += coefs[e][: len(tok1), None] * y1[: len(tok1)]
        if S2 and pieces[e] is not None:
            pe, st, ln = pieces[e]
            y2 = np.asarray(res["yT2"], np.float32)
            y2 = y2.transpose(1, 0, 2).reshape(D, S2).T
            tok2 = token_lists[pe][st : st + ln]
            out[tok2] += coefs[pe][st : st + ln, None] * y2[:ln]
    return out


# revision 22
# speedup vs baseline: 1.1299x; 1.0091x over previous
"""GPTQ-Marlin sparse MoE layer for 8 Trainium2 NeuronCores.

Strategy (expert-parallel with overflow segments, host-side dispatch):
  - Router (softmax + top-2 + renormalize) replicated with the same jax ops
    as the reference so expert selection matches bit-for-bit.
  - Per-core uniform SPMD structure of two token segments:
      seg1: S1 (=1024) token slots running one "main" expert,
      seg2: S2 (=64)  token slots running an "overflow" expert.
    Expert e's first S1 tokens go to core e's seg1; tokens beyond S1 are
    split into <=S2-sized pieces placed on cores' seg2 slots (preferring the
    expert's own core so no second weight set is actually distinct). This
    cuts per-core token capacity from pad128(max_e n_e) (=1152 for the
    reference routing) to S1+S2 (=1088); the kernel is bf16 tensor-engine
    bound, so capacity is wall-clock.
  - GPTQ int4 codes are dequantized to bf16 on the host; each core streams
    its experts' W1 [D,2F] / W2 [F,D] from HBM.
  - Device kernel per core and segment: h = x @ W1 (transposed layout),
    act = silu(gate)*up, y = act @ W2 -- bf16 matmuls, fp32 PSUM.
  - Host applies the top-k coefficients during the scatter-add combine.

Layout (activations keep tokens on the free dim; no on-device transposes):
  mm1: psum[n, t] = sum_k W1[k, n] * xT[k, t]   (lhsT = W1 as stored)
  mm2: psum[d, t] = sum_f W2[f, d] * actT[f, t] (lhsT = W2 as stored)

Schedule notes (all measured on HW traces):
  - seg1/seg2 work is interleaved chunk-by-chunk (mm1) and d-tile by d-tile
    (mm2) so the ~48 MiB/core of weight DMA streams at a flat ~140 GB/s
    instead of starving the tiny seg2 at the end.
  - seg1 weights issue on the sync queue, seg2 weights + W2b on gpsimd,
    x / y on scalar: three independent in-order DMA queues, no blocking.
  - W1 chunks and x live in SBUF as [P, 2, KO1, 512] (column/token halves)
    written by separate DMAs, so the first matmul only needs ~3 MiB landed
    and j0..j3 can run a full contraction while the rest streams.
  - silu is applied in-place on the gate PSUM bank (scalar engine only ever
    runs Silu; a dummy activation at t~0 preloads its table), and the DVE
    does the up-multiply and the mm2 PSUM->SBUF copies.
  - y is written back as bf16 (host casts to f32): halves write traffic.
"""

import numpy as np
import ml_dtypes

E, T, D, F, TOPK, GROUP = 8, 4096, 1024, 4096, 2, 128
P = 128
KO1 = D // P           # 8  k-tiles for mm1
FH = F // P            # 32 act tiles (and k-tiles for mm2)
DO = D // P            # 8  output d-tiles
NCH = (2 * F) // 1024  # 8 column chunks of W1 (0..3 gate, 4..7 up)
TC = 512               # token chunk (one PSUM bank of fp32)

LAST_RESULTS = None    # test harness introspection

_BUILD_CACHE = {}


def _route(gating_output):
    """softmax + top-k + renormalize, replicated exactly like the reference."""
    try:
        import jax
        import jax.numpy as jnp

        scores = jax.nn.softmax(jnp.asarray(gating_output, jnp.float32), axis=-1)
        topk_w, topk_ids = jax.lax.top_k(scores, TOPK)
        topk_w = topk_w / jnp.sum(topk_w, axis=-1, keepdims=True)
        return np.asarray(topk_w, np.float32), np.asarray(topk_ids)
    except Exception:
        g = np.asarray(gating_output, np.float32)
        ex = np.exp(g - g.max(axis=-1, keepdims=True))
        s = (ex / ex.sum(axis=-1, keepdims=True)).astype(np.float32)
        ids = np.argsort(-s, axis=-1, kind="stable")[:, :TOPK]
        w = np.take_along_axis(s, ids, axis=-1)
        w = (w / w.sum(axis=-1, keepdims=True)).astype(np.float32)
        return w, ids


def _dequant_bf16(q, s):
    """q: [K, N] int codes, s: [K//GROUP, N] scales -> bf16 [K, N]."""
    w = (np.asarray(q, np.float32) - 8.0) * np.repeat(
        np.asarray(s, np.float32), GROUP, axis=0
    )
    return w.astype(ml_dtypes.bfloat16)


def _build(C1, C2, native_silu=True):
    """Per-core FFN program: seg1 of C1 tokens (expert A, C1 = n*512) +
    seg2 of C2 tokens (expert B, C2 <= 512, may be 0)."""
    import concourse.mybir as mybir
    import concourse.tile as tile
    from concourse import bacc

    assert C1 % TC == 0
    NT1 = C1 // TC
    assert 0 <= C2 <= TC

    nc = bacc.Bacc("TRN2", name="moe_expert_ffn")
    bf16 = mybir.dt.bfloat16
    f32 = mybir.dt.float32

    # DRAM layouts arranged so every DMA piece is a fully contiguous block
    # (8 KiB per-partition runs -> big DGE packets, ~2.5x queue bandwidth)
    xT1 = nc.dram_tensor("xT1", [NT1, P, KO1, TC], bf16, kind="ExternalInput")
    w1a = nc.dram_tensor("w1a", [NCH, 2, P, KO1, TC], bf16, kind="ExternalInput")
    w2a = nc.dram_tensor("w2a", [DO, 2, P, FH // 2, P], bf16, kind="ExternalInput")
    yT1 = nc.dram_tensor("yT1", [P, DO, C1], bf16, kind="ExternalOutput")
    if C2:
        xT2 = nc.dram_tensor("xT2", [P, KO1, C2], bf16, kind="ExternalInput")
        w1b = nc.dram_tensor("w1b", [NCH, 2, P, KO1, TC], bf16, kind="ExternalInput")
        w2b = nc.dram_tensor("w2b", [DO, 2, P, FH // 2, P], bf16, kind="ExternalInput")
        yT2 = nc.dram_tensor("yT2", [P, DO, C2], bf16, kind="ExternalOutput")

    with tile.TileContext(nc) as tc:
        with (
            tc.tile_pool(name="xpool", bufs=1) as xpool,
            tc.tile_pool(name="w1pool", bufs=4) as w1pool,
            tc.tile_pool(name="w1bpool", bufs=2) as w1bpool,
            tc.tile_pool(name="w2pool", bufs=2) as w2pool,
            tc.tile_pool(name="actpool", bufs=1) as actpool,
            tc.tile_pool(name="miscpool", bufs=1) as miscpool,
            tc.tile_pool(name="ypool", bufs=1) as ypool,
            tc.tile_pool(name="pspool", bufs=6, space="PSUM") as pspool,
            tc.tile_pool(name="psypool", bufs=2, space="PSUM") as psypool,
        ):
            # x for seg1: one tile per 512-token half (deps are tile-granular,
            # so separate tiles let the first matmul start after ~1 MiB)
            x1 = []
            for h in range(NT1):
                x1h = xpool.tile([P, KO1, TC], bf16, tag="x1", bufs=NT1,
                                 name=f"x1h{h}")
                nc.scalar.dma_start(x1h[:], xT1[h])
                x1.append(x1h)
            # preload the ACT engine's Silu table during the DMA wait
            warm_in = miscpool.tile([P, 8], f32, tag="warm")
            warm_out = miscpool.tile([P, 8], f32, tag="warm2")
            nc.gpsimd.memset(warm_in[:], 0.0)
            if native_silu:
                nc.scalar.activation(
                    warm_out[:], warm_in[:], mybir.ActivationFunctionType.Silu
                )
            if C2:
                x2 = xpool.tile([P, KO1, C2], bf16, tag="x2")
                nc.scalar.dma_start(x2[:], xT2[:])

            act1 = actpool.tile([P, FH, C1], bf16, tag="act1")
            if C2:
                act2 = actpool.tile([P, FH, C2], bf16, tag="act2")

            def mm1_chunk(c, wg, wu, xts, act, Cw):
                """One W1 column-chunk (128*8 gate + up cols) over the token
                chunks in xts, each of width Cw (<=512). wg/wu are lists of
                two [P, KO1, TC] tiles (column-group halves)."""
                for h, xh in enumerate(xts):
                    for j in range(8):
                        cg, jj = j // 4, j % 4
                        i = c * 8 + j
                        psg = pspool.tile([P, TC], f32, tag="ps")
                        psu = pspool.tile([P, TC], f32, tag="ps")
                        for k in range(KO1):
                            nc.tensor.matmul(
                                psg[:, :Cw],
                                wg[cg][:, k, jj * P : (jj + 1) * P],
                                xh[:, k, :Cw],
                                start=(k == 0),
                                stop=(k == KO1 - 1),
                            )
                            nc.tensor.matmul(
                                psu[:, :Cw],
                                wu[cg][:, k, jj * P : (jj + 1) * P],
                                xh[:, k, :Cw],
                                start=(k == 0),
                                stop=(k == KO1 - 1),
                            )
                        # silu(gate) -> bf16 SBUF staging (DVE may read at
                        # most one PSUM operand, so the mult needs sg in SB)
                        sg = miscpool.tile([P, TC], bf16, tag="sg", bufs=1)
                        if native_silu:
                            nc.scalar.activation(
                                sg[:, :Cw], psg[:, :Cw],
                                mybir.ActivationFunctionType.Silu,
                            )
                        else:
                            nc.scalar.activation(
                                sg[:, :Cw], psg[:, :Cw],
                                mybir.ActivationFunctionType.Sigmoid,
                            )
                            nc.vector.tensor_tensor(
                                sg[:, :Cw], sg[:, :Cw], psg[:, :Cw],
                                mybir.AluOpType.mult,
                            )
                        nc.vector.tensor_tensor(
                            act[:, i, h * TC : h * TC + Cw],
                            sg[:, :Cw],
                            psu[:, :Cw],
                            mybir.AluOpType.mult,
                        )

            # ---- mm1, seg1 chunk c then seg2 chunk c ----
            for c in range(NCH // 2):
                wg, wu = [], []
                # chunk 0 is on the startup critical path: give gate-weights
                # to the sync queue and up-weights to gpsimd so the first
                # j-block only waits ~1 MiB per queue
                up_eng = nc.gpsimd if c == 0 else nc.sync
                for cg in range(2):
                    wgt = w1pool.tile([P, KO1, TC], bf16, tag="w1c", bufs=8,
                                      name=f"wg{c}{cg}")
                    nc.sync.dma_start(wgt[:], w1a[c, cg])
                    wg.append(wgt)
                    wut = w1pool.tile([P, KO1, TC], bf16, tag="w1c", bufs=8,
                                      name=f"wu{c}{cg}")
                    up_eng.dma_start(wut[:], w1a[c + 4, cg])
                    wu.append(wut)
                mm1_chunk(c, wg, wu, x1, act1, TC)
                if C2:
                    wgb, wub = [], []
                    for cg in range(2):
                        wgbt = w1bpool.tile([P, KO1, TC], bf16, tag="w1b",
                                            bufs=4, name=f"wgb{c}{cg}")
                        nc.gpsimd.dma_start(wgbt[:], w1b[c, cg])
                        wgb.append(wgbt)
                        wubt = w1bpool.tile([P, KO1, TC], bf16, tag="w1b",
                                            bufs=4, name=f"wub{c}{cg}")
                        nc.gpsimd.dma_start(wubt[:], w1b[c + 4, cg])
                        wub.append(wubt)
                    mm1_chunk(c, wgb, wub, [x2], act2, C2)

            FH2 = FH // 2

            def mm2_d(d, ws, nt, act, yT, Cw, out_engs):
                for h in range(nt):
                    psy = psypool.tile([P, TC], f32, tag="psy")
                    for k2 in range(FH):
                        nc.tensor.matmul(
                            psy[:, :Cw],
                            ws[k2 // FH2][:, k2 % FH2],
                            act[:, k2, h * TC : h * TC + Cw],
                            start=(k2 == 0),
                            stop=(k2 == FH - 1),
                        )
                    yo = ypool.tile([P, TC], bf16, tag="yo")
                    nc.vector.tensor_copy(yo[:, :Cw], psy[:, :Cw])
                    out_engs[h % len(out_engs)].dma_start(
                        yT[:, d, h * TC : h * TC + Cw], yo[:, :Cw]
                    )

            # ---- mm2, seg1 d-tile then seg2 d-tile ----
            for d in range(DO):
                wsa = []
                for g in range(2):
                    wsat = w2pool.tile([P, FH2, P], bf16, tag="w2a", bufs=4,
                                       name=f"wsa{d}{g}")
                    nc.sync.dma_start(wsat[:], w2a[d, g])
                    wsa.append(wsat)
                mm2_d(d, wsa, NT1, act1, yT1, TC, [nc.sync, nc.scalar])
                if C2:
                    wsb = []
                    for g in range(2):
                        wsbt = w2pool.tile([P, FH2, P], bf16, tag="w2b", bufs=2,
                                           name=f"wsb{d}{g}")
                        nc.gpsimd.dma_start(wsbt[:], w2b[d, g])
                        wsb.append(wsbt)
                    mm2_d(d, wsb, 1, act2, yT2, C2, [nc.scalar])
    return nc


def _pack(counts, S1=1024):
    """Choose seg2 size S2 and overflow piece placement.

    Returns (S2, pieces); pieces[core] = (expert, start, length) or None.
    Pieces are assigned to the overflowing expert's own core first (so its
    seg2 weight set is the same data), then to cores with free seg2 slots.
    """
    counts = np.asarray(counts)
    over = np.maximum(counts - S1, 0)
    for S2 in (64, 128, 192, 256, 384, 512):
        if int(np.ceil(over / S2).sum()) > E:
            continue
        pieces = [None] * E
        rest = []
        for e in range(E):
            o, st = int(over[e]), S1
            own = True
            while o > 0:
                ln = min(o, S2)
                if own and pieces[e] is None:
                    pieces[e] = (e, st, ln)
                    own = False
                else:
                    rest.append((e, st, ln))
                st += ln
                o -= ln
        free = [i for i in range(E) if pieces[i] is None]
        if len(rest) > len(free):
            continue
        for slot, pc in zip(free, rest):
            pieces[slot] = pc
        return S2, pieces
    return None, None


def kernel(x, gating_output, w1_q, w2_q, w1_scale, w2_scale):
    global LAST_RESULTS
    from concourse.bass_utils import run_bass_kernel_spmd

    x = np.asarray(x, np.float32)
    w1_q = np.asarray(w1_q)
    w2_q = np.asarray(w2_q)
    w1_scale = np.asarray(w1_scale, np.float32)
    w2_scale = np.asarray(w2_scale, np.float32)

    topk_w, topk_ids = _route(gating_output)

    token_lists, coefs = [], []
    for e in range(E):
        mask = topk_ids == e
        tok = np.nonzero(mask.any(axis=1))[0]
        cf = np.where(mask, topk_w, 0.0).sum(axis=1)[tok].astype(np.float32)
        token_lists.append(tok)
        coefs.append(cf)
    counts = np.array([len(t) for t in token_lists])

    S1 = 1024
    S2, pieces = _pack(counts, S1)
    if S2 is None:
        S1 = max(TC, int(-(-counts.max() // TC)) * TC)
        S2, pieces = 0, [None] * E

    key = (S1, S2)
    if key not in _BUILD_CACHE:
        nc = _build(S1, S2)
        nc.finalize()
        _BUILD_CACHE[key] = nc
    nc = _BUILD_CACHE[key]

    # host-side weight prep, once per expert; layouts chosen so each device
    # DMA piece is one contiguous block (see _build)
    w1h, w2h = [], []
    for e in range(E):
        w1d = _dequant_bf16(w1_q[e], w1_scale[e])   # [D, 2F]
        # [NCH, 2, P, KO1, TC]: [c, cg, p, ko, col]
        w1h.append(np.ascontiguousarray(
            w1d.reshape(KO1, P, NCH, 2, TC).transpose(2, 3, 1, 0, 4)
        ))
        w2d = _dequant_bf16(w2_q[e], w2_scale[e])   # [F, D]
        # [DO, 2, P, FH//2, P]: [d, g, p, f2, pp]
        w2h.append(np.ascontiguousarray(
            w2d.reshape(2, FH // 2, P, DO, P).transpose(3, 0, 2, 1, 4)
        ))

    def xT1_of(tok, C):
        xe = np.zeros((C, D), np.float32)
        xe[: len(tok)] = x[tok]
        # [NT1, P, KO1, TC]: [h, p, ko, tt] = xe[h*TC+tt, ko*P+p]
        return np.ascontiguousarray(
            xe.reshape(C // TC, TC, KO1, P).transpose(0, 3, 2, 1)
        ).astype(ml_dtypes.bfloat16)

    def xT2_of(tok, C):
        xe = np.zeros((C, D), np.float32)
        xe[: len(tok)] = x[tok]
        # [P, KO1, C] with xT[p, k, t] = x[t, k*P + p]
        return np.ascontiguousarray(
            xe.T.reshape(KO1, P, C).transpose(1, 0, 2)
        ).astype(ml_dtypes.bfloat16)

    in_maps = []
    for e in range(E):
        tok1 = token_lists[e][: min(counts[e], S1)]
        m = {"xT1": xT1_of(tok1, S1), "w1a": w1h[e], "w2a": w2h[e]}
        if S2:
            pc = pieces[e]
            if pc is None:
                m["xT2"] = np.zeros((P, KO1, S2), ml_dtypes.bfloat16)
                m["w1b"], m["w2b"] = w1h[e], w2h[e]
            else:
                pe, st, ln = pc
                m["xT2"] = xT2_of(token_lists[pe][st : st + ln], S2)
                m["w1b"], m["w2b"] = w1h[pe], w2h[pe]
        in_maps.append(m)

    LAST_RESULTS = run_bass_kernel_spmd(nc, in_maps, core_ids=list(range(E)))

    out = np.zeros((T, D), np.float32)
    for e in range(E):
        res = LAST_RESULTS.results[e]
        y1 = np.asarray(res["yT1"], np.float32)
        y1 = y1.transpose(1, 0, 2).reshape(D, S1).T  # [S1, D]
        tok1 = token_lists[e][: min(counts[e], S1)]
        out[tok1] += coefs[e][: len(tok1), None] * y1[: len(tok1)]
        if S2 and pieces[e] is not None:
            pe, st, ln = pieces[e]
            y2 = np.asarray(res["yT2"], np.float32)
            y2 = y2.transpose(1, 0, 2).reshape(D, S2).T
            tok2 = token_lists[pe][st : st + ln]
            out[tok2] += coefs[pe][st : st + ln, None] * y2[:ln]
    return out


# revision 27
# speedup vs baseline: 1.1471x; 1.0152x over previous
"""GPTQ-Marlin sparse MoE layer for 8 Trainium2 NeuronCores.

Strategy (expert-parallel with overflow segments, host-side dispatch):
  - Router (softmax + top-2 + renormalize) replicated with the same jax ops
    as the reference so expert selection matches bit-for-bit.
  - Per-core uniform SPMD structure of two token segments:
      seg1: S1 (=1024) token slots running one "main" expert,
      seg2: S2 (=64)  token slots running an "overflow" expert.
    Expert e's first S1 tokens go to core e's seg1; tokens beyond S1 are
    split into <=S2-sized pieces placed on cores' seg2 slots (preferring the
    expert's own core so no second weight set is actually distinct). This
    cuts per-core token capacity from pad128(max_e n_e) (=1152 for the
    reference routing) to S1+S2 (=1088); the kernel is bf16 tensor-engine
    bound, so capacity is wall-clock.
  - GPTQ int4 codes are dequantized to bf16 on the host; each core streams
    its experts' W1 [D,2F] / W2 [F,D] from HBM.
  - Device kernel per core and segment: h = x @ W1 (transposed layout),
    act = silu(gate)*up, y = act @ W2 -- bf16 matmuls, fp32 PSUM.
  - Host applies the top-k coefficients during the scatter-add combine.

Layout (activations keep tokens on the free dim; no on-device transposes):
  mm1: psum[n, t] = sum_k W1[k, n] * xT[k, t]   (lhsT = W1 as stored)
  mm2: psum[d, t] = sum_f W2[f, d] * actT[f, t] (lhsT = W2 as stored)

Schedule notes (all measured on HW traces):
  - seg1/seg2 work is interleaved chunk-by-chunk (mm1) and d-tile by d-tile
    (mm2) so the ~48 MiB/core of weight DMA streams at a flat ~140 GB/s
    instead of starving the tiny seg2 at the end.
  - seg1 weights issue on the sync queue, seg2 weights + W2b on gpsimd,
    x / y on scalar: three independent in-order DMA queues, no blocking.
  - W1 chunks and x live in SBUF as [P, 2, KO1, 512] (column/token halves)
    written by separate DMAs, so the first matmul only needs ~3 MiB landed
    and j0..j3 can run a full contraction while the rest streams.
  - silu is applied in-place on the gate PSUM bank (scalar engine only ever
    runs Silu; a dummy activation at t~0 preloads its table), and the DVE
    does the up-multiply and the mm2 PSUM->SBUF copies.
  - y is written back as bf16 (host casts to f32): halves write traffic.
"""

import numpy as np
import ml_dtypes

E, T, D, F, TOPK, GROUP = 8, 4096, 1024, 4096, 2, 128
P = 128
KO1 = D // P           # 8  k-tiles for mm1
FH = F // P            # 32 act tiles (and k-tiles for mm2)
DO = D // P            # 8  output d-tiles
NCH = (2 * F) // 1024  # 8 column chunks of W1 (0..3 gate, 4..7 up)
TC = 512               # token chunk (one PSUM bank of fp32)

LAST_RESULTS = None    # test harness introspection

_BUILD_CACHE = {}


def _route(gating_output):
    """softmax + top-k + renormalize, replicated exactly like the reference."""
    try:
        import jax
        import jax.numpy as jnp

        scores = jax.nn.softmax(jnp.asarray(gating_output, jnp.float32), axis=-1)
        topk_w, topk_ids = jax.lax.top_k(scores, TOPK)
        topk_w = topk_w / jnp.sum(topk_w, axis=-1, keepdims=True)
        return np.asarray(topk_w, np.float32), np.asarray(topk_ids)
    except Exception:
        g = np.asarray(gating_output, np.float32)
        ex = np.exp(g - g.max(axis=-1, keepdims=True))
        s = (ex / ex.sum(axis=-1, keepdims=True)).astype(np.float32)
        ids = np.argsort(-s, axis=-1, kind="stable")[:, :TOPK]
        w = np.take_along_axis(s, ids, axis=-1)
        w = (w / w.sum(axis=-1, keepdims=True)).astype(np.float32)
        return w, ids


def _dequant_bf16(q, s):
    """q: [K, N] int codes, s: [K//GROUP, N] scales -> bf16 [K, N]."""
    w = (np.asarray(q, np.float32) - 8.0) * np.repeat(
        np.asarray(s, np.float32), GROUP, axis=0
    )
    return w.astype(ml_dtypes.bfloat16)


def _build(C1, C2, native_silu=True):
    """Per-core FFN program: seg1 of C1 tokens (expert A, C1 = n*512) +
    seg2 of C2 tokens (expert B, C2 <= 512, may be 0)."""
    import concourse.mybir as mybir
    import concourse.tile as tile
    from concourse import bacc

    assert C1 % TC == 0
    NT1 = C1 // TC
    assert 0 <= C2 <= TC

    nc = bacc.Bacc("TRN2", name="moe_expert_ffn")
    bf16 = mybir.dt.bfloat16
    f32 = mybir.dt.float32

    # DRAM layouts arranged so every DMA piece is a fully contiguous block
    # (8 KiB per-partition runs -> big DGE packets, ~2.5x queue bandwidth)
    xT1 = nc.dram_tensor("xT1", [NT1, P, KO1, TC], bf16, kind="ExternalInput")
    w1a = nc.dram_tensor("w1a", [NCH, 8, P, KO1, P], bf16, kind="ExternalInput")
    w2a = nc.dram_tensor("w2a", [DO, 2, P, FH // 2, P], bf16, kind="ExternalInput")
    yT1 = nc.dram_tensor("yT1", [P, DO, C1], bf16, kind="ExternalOutput")
    if C2:
        xT2 = nc.dram_tensor("xT2", [P, KO1, C2], bf16, kind="ExternalInput")
        w1b = nc.dram_tensor("w1b", [NCH, 8, P, KO1, P], bf16, kind="ExternalInput")
        w2b = nc.dram_tensor("w2b", [DO, 2, P, FH // 2, P], bf16, kind="ExternalInput")
        yT2 = nc.dram_tensor("yT2", [P, DO, C2], bf16, kind="ExternalOutput")

    with tile.TileContext(nc) as tc:
        with (
            tc.tile_pool(name="xpool", bufs=1) as xpool,
            tc.tile_pool(name="w1pool", bufs=4) as w1pool,
            tc.tile_pool(name="w1bpool", bufs=2) as w1bpool,
            tc.tile_pool(name="w2pool", bufs=2) as w2pool,
            tc.tile_pool(name="actpool", bufs=1) as actpool,
            tc.tile_pool(name="miscpool", bufs=1) as miscpool,
            tc.tile_pool(name="ypool", bufs=1) as ypool,
            tc.tile_pool(name="pspool", bufs=6, space="PSUM") as pspool,
            tc.tile_pool(name="psypool", bufs=2, space="PSUM") as psypool,
        ):
            # x for seg1: one tile per (512-token half, k-half) so the first
            # k-sweep's dependencies are ~0.5 MiB tiles spread over two
            # queues (deps are tile-granular)
            KH = KO1 // 2
            x1 = []
            for h in range(NT1):
                pieces = []
                for kh in range(2):
                    xp = xpool.tile([P, KH, TC], bf16, tag="x1", bufs=2 * NT1,
                                    name=f"x1h{h}k{kh}")
                    eng = nc.scalar if (h > 0 or kh == 0) else nc.sync
                    eng.dma_start(xp[:], xT1[h, :, kh * KH : (kh + 1) * KH])
                    pieces.append(xp)
                x1.append(pieces)
            # preload the ACT engine's Silu table during the DMA wait
            warm_in = miscpool.tile([P, 8], f32, tag="warm")
            warm_out = miscpool.tile([P, 8], f32, tag="warm2")
            nc.gpsimd.memset(warm_in[:], 0.0)
            if native_silu:
                nc.scalar.activation(
                    warm_out[:], warm_in[:], mybir.ActivationFunctionType.Silu
                )
            if C2:
                x2 = xpool.tile([P, KO1, C2], bf16, tag="x2")
                nc.scalar.dma_start(x2[:], xT2[:])

            act1 = actpool.tile([P, FH, C1], bf16, tag="act1")
            if C2:
                act2 = actpool.tile([P, FH, C2], bf16, tag="act2")

            def mm1_chunk(c, wg, wu, nt, xfn, act, Cw):
                """One W1 column-chunk (128*8 gate + up cols) over nt token
                chunks of width Cw (<=512). wg/wu: lists of 8 per-j [P, KO1,
                P] tiles; xfn(h, k) -> rhs AP [P, Cw] for token chunk h."""
                for h in range(nt):
                    for j in range(8):
                        i = c * 8 + j
                        psg = pspool.tile([P, TC], f32, tag="ps")
                        psu = pspool.tile([P, TC], f32, tag="ps")
                        for k in range(KO1):
                            xk = xfn(h, k)
                            nc.tensor.matmul(
                                psg[:, :Cw],
                                wg[j][:, k],
                                xk,
                                start=(k == 0),
                                stop=(k == KO1 - 1),
                            )
                            nc.tensor.matmul(
                                psu[:, :Cw],
                                wu[j][:, k],
                                xk,
                                start=(k == 0),
                                stop=(k == KO1 - 1),
                            )
                        # silu(gate) -> bf16 SBUF staging (DVE may read at
                        # most one PSUM operand, so the mult needs sg in SB)
                        sg = miscpool.tile([P, TC], bf16, tag="sg", bufs=1)
                        if native_silu:
                            nc.scalar.activation(
                                sg[:, :Cw], psg[:, :Cw],
                                mybir.ActivationFunctionType.Silu,
                            )
                        else:
                            nc.scalar.activation(
                                sg[:, :Cw], psg[:, :Cw],
                                mybir.ActivationFunctionType.Sigmoid,
                            )
                            nc.vector.tensor_tensor(
                                sg[:, :Cw], sg[:, :Cw], psg[:, :Cw],
                                mybir.AluOpType.mult,
                            )
                        nc.vector.tensor_tensor(
                            act[:, i, h * TC : h * TC + Cw],
                            sg[:, :Cw],
                            psu[:, :Cw],
                            mybir.AluOpType.mult,
                        )

            # ---- mm1, seg1 chunk c then seg2 chunk c ----
            def x1fn(h, k):
                return x1[h][k // KH][:, k % KH]

            def x2fn(h, k):
                return x2[:, k]

            for c in range(NCH // 2):
                wg, wu = [], []
                # per-j weight tiles: the first k-sweep only depends on
                # ~0.25 MiB per operand. gate on sync, up on gpsimd (c0) /
                # sync (rest)
                up_eng = nc.gpsimd if c == 0 else nc.sync
                for j in range(8):
                    cg, jj = j // 4, j % 4
                    wgt = w1pool.tile([P, KO1, P], bf16, tag="w1c", bufs=32,
                                      name=f"wg{c}{j}")
                    nc.sync.dma_start(wgt[:], w1a[c, j])
                    wg.append(wgt)
                    wut = w1pool.tile([P, KO1, P], bf16, tag="w1c", bufs=32,
                                      name=f"wu{c}{j}")
                    up_eng.dma_start(wut[:], w1a[c + 4, j])
                    wu.append(wut)
                mm1_chunk(c, wg, wu, NT1, x1fn, act1, TC)
                if C2:
                    wgb, wub = [], []
                    for j in range(8):
                        cg, jj = j // 4, j % 4
                        wgbt = w1bpool.tile([P, KO1, P], bf16, tag="w1b",
                                            bufs=16, name=f"wgb{c}{j}")
                        nc.gpsimd.dma_start(wgbt[:], w1b[c, j])
                        wgb.append(wgbt)
                        wubt = w1bpool.tile([P, KO1, P], bf16, tag="w1b",
                                            bufs=16, name=f"wub{c}{j}")
                        nc.gpsimd.dma_start(wubt[:], w1b[c + 4, j])
                        wub.append(wubt)
                    mm1_chunk(c, wgb, wub, 1, x2fn, act2, C2)

            FH2 = FH // 2

            def mm2_d(d, ws, nt, act, yT, Cw, out_engs):
                for h in range(nt):
                    psy = psypool.tile([P, TC], f32, tag="psy")
                    for k2 in range(FH):
                        nc.tensor.matmul(
                            psy[:, :Cw],
                            ws[k2 // FH2][:, k2 % FH2],
                            act[:, k2, h * TC : h * TC + Cw],
                            start=(k2 == 0),
                            stop=(k2 == FH - 1),
                        )
                    yo = ypool.tile([P, TC], bf16, tag="yo")
                    nc.vector.tensor_copy(yo[:, :Cw], psy[:, :Cw])
                    out_engs[h % len(out_engs)].dma_start(
                        yT[:, d, h * TC : h * TC + Cw], yo[:, :Cw]
                    )

            # ---- mm2, seg1 d-tile then seg2 d-tile ----
            for d in range(DO):
                wsa = []
                for g in range(2):
                    wsat = w2pool.tile([P, FH2, P], bf16, tag="w2a", bufs=4,
                                       name=f"wsa{d}{g}")
                    nc.sync.dma_start(wsat[:], w2a[d, g])
                    wsa.append(wsat)
                mm2_d(d, wsa, NT1, act1, yT1, TC, [nc.sync, nc.scalar])
                if C2:
                    wsb = []
                    for g in range(2):
                        wsbt = w2pool.tile([P, FH2, P], bf16, tag="w2b", bufs=2,
                                           name=f"wsb{d}{g}")
                        nc.gpsimd.dma_start(wsbt[:], w2b[d, g])
                        wsb.append(wsbt)
                    mm2_d(d, wsb, 1, act2, yT2, C2, [nc.scalar])
    return nc


def _pack(counts, S1=1024):
    """Choose seg2 size S2 and overflow piece placement.

    Returns (S2, pieces); pieces[core] = (expert, start, length) or None.
    Pieces are assigned to the overflowing expert's own core first (so its
    seg2 weight set is the same data), then to cores with free seg2 slots.
    """
    counts = np.asarray(counts)
    over = np.maximum(counts - S1, 0)
    for S2 in (64, 128, 192, 256, 384, 512):
        if int(np.ceil(over / S2).sum()) > E:
            continue
        pieces = [None] * E
        rest = []
        for e in range(E):
            o, st = int(over[e]), S1
            own = True
            while o > 0:
                ln = min(o, S2)
                if own and pieces[e] is None:
                    pieces[e] = (e, st, ln)
                    own = False
                else:
                    rest.append((e, st, ln))
                st += ln
                o -= ln
        free = [i for i in range(E) if pieces[i] is None]
        if len(rest) > len(free):
            continue
        for slot, pc in zip(free, rest):
            pieces[slot] = pc
        return S2, pieces
    return None, None


def kernel(x, gating_output, w1_q, w2_q, w1_scale, w2_scale):
    global LAST_RESULTS
    from concourse.bass_utils import run_bass_kernel_spmd

    x = np.asarray(x, np.float32)
    w1_q = np.asarray(w1_q)
    w2_q = np.asarray(w2_q)
    w1_scale = np.asarray(w1_scale, np.float32)
    w2_scale = np.asarray(w2_scale, np.float32)

    topk_w, topk_ids = _route(gating_output)

    token_lists, coefs = [], []
    for e in range(E):
        mask = topk_ids == e
        tok = np.nonzero(mask.any(axis=1))[0]
        cf = np.where(mask, topk_w, 0.0).sum(axis=1)[tok].astype(np.float32)
        token_lists.append(tok)
        coefs.append(cf)
    counts = np.array([len(t) for t in token_lists])

    S1 = 1024
    S2, pieces = _pack(counts, S1)
    if S2 is None:
        S1 = max(TC, int(-(-counts.max() // TC)) * TC)
        S2, pieces = 0, [None] * E

    key = (S1, S2)
    if key not in _BUILD_CACHE:
        nc = _build(S1, S2)
        nc.finalize()
        _BUILD_CACHE[key] = nc
    nc = _BUILD_CACHE[key]

    # host-side weight prep, once per expert; layouts chosen so each device
    # DMA piece is one contiguous block (see _build)
    w1h, w2h = [], []
    for e in range(E):
        w1d = _dequant_bf16(w1_q[e], w1_scale[e])   # [D, 2F]
        # [NCH, 8, P, KO1, P]: [c, j, p, ko, col]
        w1h.append(np.ascontiguousarray(
            w1d.reshape(KO1, P, NCH, 8, P).transpose(2, 3, 1, 0, 4)
        ))
        w2d = _dequant_bf16(w2_q[e], w2_scale[e])   # [F, D]
        # [DO, 2, P, FH//2, P]: [d, g, p, f2, pp]
        w2h.append(np.ascontiguousarray(
            w2d.reshape(2, FH // 2, P, DO, P).transpose(3, 0, 2, 1, 4)
        ))

    def xT1_of(tok, C):
        xe = np.zeros((C, D), np.float32)
        xe[: len(tok)] = x[tok]
        # [NT1, P, KO1, TC]: [h, p, ko, tt] = xe[h*TC+tt, ko*P+p]
        return np.ascontiguousarray(
            xe.reshape(C // TC, TC, KO1, P).transpose(0, 3, 2, 1)
        ).astype(ml_dtypes.bfloat16)

    def xT2_of(tok, C):
        xe = np.zeros((C, D), np.float32)
        xe[: len(tok)] = x[tok]
        # [P, KO1, C] with xT[p, k, t] = x[t, k*P + p]
        return np.ascontiguousarray(
            xe.T.reshape(KO1, P, C).transpose(1, 0, 2)
        ).astype(ml_dtypes.bfloat16)

    in_maps = []
    for e in range(E):
        tok1 = token_lists[e][: min(counts[e], S1)]
        m = {"xT1": xT1_of(tok1, S1), "w1a": w1h[e], "w2a": w2h[e]}
        if S2:
            pc = pieces[e]
            if pc is None:
                m["xT2"] = np.zeros((P, KO1, S2), ml_dtypes.bfloat16)
                m["w1b"], m["w2b"] = w1h[e], w2h[e]
            else:
                pe, st, ln = pc
                m["xT2"] = xT2_of(token_lists[pe][st : st + ln], S2)
                m["w1b"], m["w2b"] = w1h[pe], w2h[pe]
        in_maps.append(m)

    LAST_RESULTS = run_bass_kernel_spmd(nc, in_maps, core_ids=list(range(E)))

    out = np.zeros((T, D), np.float32)
    for e in range(E):
        res = LAST_RESULTS.results[e]
        y1 = np.asarray(res["yT1"], np.float32)
        y1 = y1.transpose(1, 0, 2).reshape(D, S1).T  # [S1, D]
        tok1 = token_lists[e][: min(counts[e], S1)]
        out[tok1] += coefs[e][: len(tok1), None] * y1[: len(tok1)]
        if S2 and pieces[e] is not None:
            pe, st, ln = pieces[e]
            y2 = np.asarray(res["yT2"], np.float32)
            y2 = y2.transpose(1, 0, 2).reshape(D, S2).T
            tok2 = token_lists[pe][st : st + ln]
            out[tok2] += coefs[pe][st : st + ln, None] * y2[:ln]
    return out
